# revision 1
# baseline (speedup 1.0000x reference)
"""Trainium2 Bass kernel for nn_MirrorDescentLinear.

Reference computation:
    w[o,i] = (e1 - e0) / (1 + e0 + e1)            (softmax(+1) - softmax(-1))
    w *= bf16(scales)[o, i//128]                   (per-group scale)
    w *= mask[o,i]                                 (0/1 int mask)
    y = x @ w.T                                    (f32, [8192,4096]@[4096,4096].T)

Sharding (8 cores): tensor-parallel 4-way on out_features x 2-way on tokens.
Each core computes y[t_half, o_quarter] from logits/scales/mask[o_quarter]
and xT[:, t_half]. The host pre-transposes x once (layout-only) so the
contraction dim I lands on SBUF partitions without any on-chip transpose of x.

Per-core device program:
  phase A (weights): exp on ScalarE; 1/d as exp(-ln d) on ScalarE; e1-e0,
    int-mask multiply, stride-0-broadcast group-scale multiply and recip
    multiply on VectorE; w tiles transposed on TensorE (4 per PSUM bank,
    single-copy evacuation) into resident wT[i, o] float32r tiles, one per
    512-wide i-chunk so phase B can start consuming early chunks.
  phase B (matmul): float32r matmuls (full-rate at N=512, FP22 mantissa)
    accumulating over 32 i-chunks into PSUM; VectorE evacuates, DMA stores y.

Measured on trn2 (single-core loop bench, host I/O excluded): ~740 us/core.
"""

import json
import sys

sys.path.insert(0, "/opt/trn_rl_repo")

import numpy as np

import concourse.bass as bass
import concourse.tile as tile
from concourse import mybir
from concourse.bass_utils import run_bass_kernel_spmd
from concourse.masks import make_identity
from concourse.tile_scheduler import N_PROCS
from concourse.vector_clock import ScopedClock, VectorClock

# ---------------------------------------------------------------------------
# Compatibility patches for the bundled walrus (accepts at most ONE sync wait
# per instruction; rejects any wait on Drain).
# ---------------------------------------------------------------------------


def _drain_and_barrier_split(self, tick_clock, wait_clock):
    g = tick_clock.global_clock
    for p in range(N_PROCS):
        tick = g.peek_next(p) - 1
        if tick <= 0:
            continue
        vc = VectorClock()
        vc.require_at_least(p, tick)
        nop = self.nc.sync.nop(nofuse=True, hint="tail_wait_split")
        wait_clock.add_sem_waits(nop.ins, ScopedClock({None: vc}))

    self.nc.sync.drain()

    self.nc.all_engine_barrier()
    assert self.sems is not None
    popped = self.nc._tile_sem_poison_stack.pop()
    assert popped is self._sem_poison
    self.nc.clear_and_free_semaphores(list(self.sems.allocated().values()))
    self.nc.all_engine_barrier()


_orig_to_json_bytes = bass.Bass.to_json_bytes
_split_ctr = [0]


def _to_json_bytes_split(self):
    raw = _orig_to_json_bytes(self)
    m = json.loads(raw)
    changed = False
    for fn in m.get("functions", []):
        for blk in fn.get("blocks", []):
            insts = blk.get("instructions")
            if not insts:
                continue
            out = []
            for inst in insts:
                si = inst.get("sync_info")
                ow = (si or {}).get("on_wait") or []
                eng = inst.get("engine")
                if len(ow) > 1 and eng:
                    changed = True
                    for w in ow[:-1]:
                        _split_ctr[0] += 1
                        nop = {
                            "engine": eng,
                            "ins": [],
                            "outs": [],
                            "name": f"I-wsplit-{_split_ctr[0]}",
                            "opcode": "NoOp",
                            "sync_info": {"on_update": [], "on_wait": [w]},
                            "text_hint": "wait_split",
                        }
                        if inst.get("debug") is not None:
                            nop["debug"] = inst["debug"]
                        out.append(nop)
                    si["on_wait"] = [ow[-1]]
                out.append(inst)
            blk["instructions"] = out
    return json.dumps(m).encode() if changed else raw


_patched = False


def _install_patches():
    global _patched
    if _patched:
        return
    tile.TileContext._drain_and_barrier = _drain_and_barrier_split
    bass.Bass.to_json_bytes = _to_json_bytes_split
    # Calibrate the scheduler's cost model to measured HW rates: ACT and DVE
    # run slower than the stock model (per-op overheads), which otherwise
    # makes the static PE instruction stream stall on weight-production.
    from concourse.hw_specs import TRN2Spec

    TRN2Spec.CYCLE_T = {
        **TRN2Spec.CYCLE_T,
        mybir.EngineType.DVE: 1e9 / 0.96e9 * 1.4,
        mybir.EngineType.Activation: 1e9 / 1.2e9 * 1.9,
    }
    _patched = True


# ---------------------------------------------------------------------------
# Problem constants (hardcoded per contest rules)
# ---------------------------------------------------------------------------

T_FULL, O_FULL, I_FULL, G = 8192, 4096, 4096, 128
N_OSH, N_TSH = 4, 2  # o-quarters x t-halves = 8 cores
O_SH, T_SH = O_FULL // N_OSH, T_FULL // N_TSH  # 1024, 4096
N_OC = O_SH // 512  # 512-wide output chunks per core (2)
NK = I_FULL // 128  # 32 contraction chunks of 128
N_IC = 8  # i-chunks of 512 in phase A
N_OB = O_SH // 128  # 8 o-blocks per core
N_TT = T_SH // 128  # 32 t-tiles per core

f32 = mybir.dt.float32
f32r = mybir.dt.float32r
i32 = mybir.dt.int32
bf16 = mybir.dt.bfloat16

AF = mybir.ActivationFunctionType
ALU = mybir.AluOpType


def build_program(bench_loop=None, phases=("A", "B")) -> bass.Bass:
    _install_patches()
    nc = bass.Bass()
    if bench_loop is None:
        xT = nc.declare_dram_parameter("xT", [I_FULL, T_SH], f32r, isOutput=False)
        logits = nc.declare_dram_parameter("logits", [O_SH, I_FULL, 2], f32, isOutput=False)
        scales = nc.declare_dram_parameter("scales", [O_SH, I_FULL // G], f32, isOutput=False)
        mask = nc.declare_dram_parameter("mask", [O_SH, I_FULL], i32, isOutput=False)
        y = nc.declare_dram_parameter("y", [T_SH, O_SH], f32, isOutput=True)
    else:
        # timing-bench build: no host I/O of the big tensors, body looped
        dummy = nc.declare_dram_parameter("bench_in", [128, 1], f32, isOutput=False)
        dout = nc.declare_dram_parameter("bench_out", [128, 1], f32, isOutput=True)
        xT = nc.dram_tensor("xT_i", [I_FULL, T_SH], f32r)
        logits = nc.dram_tensor("logits_i", [O_SH, I_FULL, 2], f32)
        scales = nc.dram_tensor("scales_i", [O_SH, I_FULL // G], f32)
        mask = nc.dram_tensor("mask_i", [O_SH, I_FULL], i32)
        y = nc.dram_tensor("y_i", [T_SH, O_SH], f32)

    xT_t = xT.rearrange("(k p) t -> p k t", p=128)  # [128, NK, T_SH]
    scales_t = scales.rearrange("(ob p) g -> p ob g", p=128)  # [128, N_OB, 32]

    with tile.TileContext(nc) as tc:
        with (
            tc.tile_pool(name="persist", bufs=1) as persist,
            tc.tile_pool(name="wt", bufs=1) as wt_pool,
            tc.tile_pool(name="wa", bufs=2) as wa,
            tc.tile_pool(name="xin", bufs=2) as xin,
            tc.tile_pool(name="yout", bufs=2) as yout,
            tc.tile_pool(name="psa", bufs=3, space="PSUM") as psa,
            tc.tile_pool(name="psb", bufs=4, space="PSUM") as psb,
        ):
            ident = persist.tile([128, 128], f32)
            make_identity(nc, ident)

            # scales for all o-blocks, rounded through bf16 once
            s_raw = persist.tile([128, N_OB, 32], f32, tag="sraw")
            nc.sync.dma_start(out=s_raw, in_=scales_t)
            s_bf = persist.tile([128, N_OB, 32], bf16, tag="sbf")
            nc.vector.tensor_copy(out=s_bf, in_=s_raw)
            s_r = persist.tile([128, N_OB, 32], f32, tag="sr")
            nc.vector.tensor_copy(out=s_r, in_=s_bf)

            # resident transposed weights, one tile per 512-wide i-chunk
            # (layout [128 part, 4 k-quarters, O_SH])
            wT = [
                wt_pool.tile([128, 4, O_SH], f32r, tag=f"wT{ic}", name=f"wT{ic}")
                for ic in range(N_IC)
            ]

            if "A" not in phases:
                for ic in range(N_IC):
                    nc.vector.memset(wT[ic].bitcast(f32), 0.0)

            if bench_loop is not None:
                dcp = persist.tile([128, 1], f32, tag="dcp")
                nc.sync.dma_start(out=dcp, in_=dummy[:, :])
                nc.sync.dma_start(out=dout[:, :], in_=dcp)

            import contextlib

            loop_cm = (
                tc.For_i(0, bench_loop, 1)
                if bench_loop is not None
                else contextlib.nullcontext()
            )
            with loop_cm:
                _emit_body(nc, tc, wa, xin, yout, psa, psb, wT, ident, s_r,
                           xT_t, logits, mask, y, phases)

    if bench_loop is not None:
        # tie dummy IO so the program has external IO
        pass
    return nc


def _emit_body(nc, tc, wa, xin, yout, psa, psb, wT, ident, s_r, xT_t, logits, mask, y, phases=("A", "B")):
            # ---- phase A: weights (ic-outer so wT[ic] complete early) ----
            for ic in range(N_IC if "A" in phases else 0):
                for ob in range(N_OB):
                    L = wa.tile([128, 512, 2], f32, tag="L", bufs=4)
                    nc.sync.dma_start(
                        out=L, in_=logits[ob * 128 : (ob + 1) * 128, ic * 512 : (ic + 1) * 512, :]
                    )
                    M = wa.tile([128, 512], i32, tag="M", bufs=4)
                    nc.sync.dma_start(
                        out=M, in_=mask[ob * 128 : (ob + 1) * 128, ic * 512 : (ic + 1) * 512]
                    )
                    # E = exp(logits), in place
                    Lf = L.rearrange("p i s -> p (i s)")
                    nc.scalar.activation(out=Lf, in_=Lf, func=AF.Exp)
                    # D = e0 + 1 + e1
                    D = wa.tile([128, 512], f32, tag="D")
                    nc.vector.scalar_tensor_tensor(
                        out=D, in0=L[:, :, 0], scalar=1.0, in1=L[:, :, 1],
                        op0=ALU.add, op1=ALU.add,
                    )
                    # D <- 1/D via exp(-ln D)  (ScalarE; DVE reciprocal is slow)
                    if "norecip" not in phases:
                        nc.scalar.activation(out=D, in_=D, func=AF.Ln)
                        nc.scalar.activation(out=D, in_=D, func=AF.Exp, scale=-1.0)
                    # N = e1 - e0
                    N = wa.tile([128, 512], f32, tag="N")
                    nc.vector.tensor_tensor(
                        out=N, in0=L[:, :, 1], in1=L[:, :, 0], op=ALU.subtract
                    )
                    # N <- N * mask  (DVE auto-casts the int32 operand)
                    nc.vector.tensor_tensor(out=N, in0=N, in1=M, op=ALU.mult)
                    # N <- N * s_g  (stride-0 broadcast of the 4 group scales)
                    s_sl = s_r[:, ob, ic * 4 : (ic + 1) * 4]
                    s_bc = bass.AP(
                        tensor=s_sl.tensor,
                        offset=s_sl.offset,
                        ap=[s_sl.ap[0], s_sl.ap[1], [0, 128]],
                    )
                    N3 = N.rearrange("p (g c) -> p g c", g=4)
                    nc.vector.tensor_tensor(out=N3, in0=N3, in1=s_bc, op=ALU.mult)
                    # N <- N * (1/D)
                    nc.vector.tensor_tensor(out=N, in0=N, in1=D, op=ALU.mult)
                    # transpose 4x 128x128 blocks into one PSUM bank, then
                    # evacuate all four with a single ScalarE copy
                    pt = psa.tile([128, 512], f32, tag="pt")
                    for q in range(4):
                        nc.tensor.transpose(
                            out=pt[:, q * 128 : (q + 1) * 128],
                            in_=N[:, q * 128 : (q + 1) * 128],
                            identity=ident,
                        )
                    nc.vector.tensor_copy(
                        out=wT[ic][:, :, ob * 128 : (ob + 1) * 128],
                        in_=pt.rearrange("p (q c) -> p q c", q=4),
                    )

            # ---- phase B: y[t, o] = sum_k xT[k,t].T @ wT[k][:, o] ----
            # Accumulation is split into 4 quarter-groups of 8 k-chunks
            # (2 i-chunks each) so PSUM tiles close and recycle as soon as
            # early weight chunks exist -- that lets phase B's matmuls fill
            # the TensorE pipe while later weights are still being built.
            for tt in range(N_TT if "B" in phases else 0):
                xTt = xin.tile([128, NK, 128], f32r, tag="xTt")
                nc.sync.dma_start(
                    out=xTt, in_=xT_t[:, :, tt * 128 : (tt + 1) * 128]
                )
                pbs = [psb.tile([128, 512], f32, tag="pb", name=f"pb{oc}") for oc in range(N_OC)]
                for k in range(NK):
                    ic, q = divmod(k, 4)
                    for oc in range(N_OC):
                        nc.tensor.matmul(
                            out=pbs[oc],
                            lhsT=xTt[:, k, :],
                            rhs=wT[ic][:, q, oc * 512 : (oc + 1) * 512],
                            start=(k == 0),
                            stop=(k == NK - 1),
                        )
                y_sb = yout.tile([128, O_SH], f32, tag="ysb", bufs=3)
                for oc in range(N_OC):
                    nc.vector.tensor_copy(
                        out=y_sb[:, oc * 512 : (oc + 1) * 512], in_=pbs[oc]
                    )
                nc.sync.dma_start(
                    out=y[tt * 128 : (tt + 1) * 128, :], in_=y_sb
                )


_prog = None


def _get_program() -> bass.Bass:
    global _prog
    if _prog is None:
        _prog = build_program()
    return _prog


def kernel(x, logits, scales, mask):
    nc = _get_program()
    x = np.asarray(x, dtype=np.float32)
    logits = np.asarray(logits, dtype=np.float32)
    scales = np.asarray(scales, dtype=np.float32)
    mask_i = np.asarray(mask, dtype=np.int32)

    xT = np.ascontiguousarray(x.T)  # [I, T]
    in_maps = []
    for c in range(8):
        th, oq = divmod(c, N_OSH)
        in_maps.append(
            {
                "xT": np.ascontiguousarray(xT[:, th * T_SH : (th + 1) * T_SH]),
                "logits": np.ascontiguousarray(logits[oq * O_SH : (oq + 1) * O_SH]),
                "scales": np.ascontiguousarray(scales[oq * O_SH : (oq + 1) * O_SH]),
                "mask": np.ascontiguousarray(mask_i[oq * O_SH : (oq + 1) * O_SH]),
            }
        )
    res = run_bass_kernel_spmd(nc, in_maps, core_ids=list(range(8)))
    yf = np.empty((T_FULL, O_FULL), dtype=np.float32)
    for c in range(8):
        th, oq = divmod(c, N_OSH)
        yf[th * T_SH : (th + 1) * T_SH, oq * O_SH : (oq + 1) * O_SH] = res.results[c][
            "y"
        ]
    return yf



# revision 4
# speedup vs baseline: 1.5526x; 1.5526x over previous
"""Trainium2 Bass kernel for nn_MirrorDescentLinear.

Reference computation:
    w[o,i] = (e1 - e0) / (1 + e0 + e1)            (softmax(+1) - softmax(-1))
    w *= bf16(scales)[o, i//128]                   (per-group scale)
    w *= mask[o,i]                                 (0/1 int mask)
    y = x @ w.T                                    (f32, [8192,4096]@[4096,4096].T)

Sharding (8 cores): tensor-parallel 8-way on out_features (O_SH=512/core),
tokens replicated. The host pre-transposes logits/mask to [I, O] layout and
x to i-major tiles (layout + fp16 cast only), so the whole weight pipeline
runs elementwise in the TRANSPOSED layout and produces wT[i, o] directly --
no PE transposes, no PSUM use in phase A. PE does nothing but the 2048
N=512 fp16 matmuls (the 437us roofline at 2.4GHz).

Per-core device program (fp16 math, f32 accumulation):
  phase A (weights, per 128-wide i-chunk kc): exp on ScalarE (fp16);
    D' = e0+e1 (f32), N = e1-e0, N*mask on GpSimd/Pool; 1/(1+D') as
    exp(-ln(D'+1)) on ScalarE (ln bias folds the +1, Ln/Exp share one
    activation table -> no table reloads); w = N*r*s on DVE (fp16 2x mode,
    the group scale is host-replicated across partitions so it is a packed
    operand). wT[kc] tiles are resident fp16 [128,512].
  phase B (matmul, 3 k-splits 4/12/16): per token-tile tt accumulate the
    split's k-range in PSUM; split 0 copies PSUM -> y_sb[tt] (fp16), split 1
    adds into y_sb, split 2 adds and stores f32 y. The early small split
    lets PE start ~20us in; SBUF partial-accumulation frees PSUM so all 8
    banks pipeline phase B.
"""

import json
import sys

sys.path.insert(0, "/opt/trn_rl_repo")

import numpy as np

import concourse.bass as bass
import concourse.tile as tile
from concourse import mybir
from concourse.bass_utils import run_bass_kernel_spmd
from concourse.tile_scheduler import N_PROCS
from concourse.vector_clock import ScopedClock, VectorClock

# ---------------------------------------------------------------------------
# Compatibility patches for the bundled walrus (accepts at most ONE sync wait
# per instruction; rejects any wait on Drain).
# ---------------------------------------------------------------------------


def _drain_and_barrier_split(self, tick_clock, wait_clock):
    g = tick_clock.global_clock
    for p in range(N_PROCS):
        tick = g.peek_next(p) - 1
        if tick <= 0:
            continue
        vc = VectorClock()
        vc.require_at_least(p, tick)
        nop = self.nc.sync.nop(nofuse=True, hint="tail_wait_split")
        wait_clock.add_sem_waits(nop.ins, ScopedClock({None: vc}))

    self.nc.sync.drain()

    self.nc.all_engine_barrier()
    assert self.sems is not None
    popped = self.nc._tile_sem_poison_stack.pop()
    assert popped is self._sem_poison
    self.nc.clear_and_free_semaphores(list(self.sems.allocated().values()))
    self.nc.all_engine_barrier()


_orig_to_json_bytes = bass.Bass.to_json_bytes
_split_ctr = [0]


def _to_json_bytes_split(self):
    raw = _orig_to_json_bytes(self)
    m = json.loads(raw)
    changed = False
    for fn in m.get("functions", []):
        for blk in fn.get("blocks", []):
            insts = blk.get("instructions")
            if not insts:
                continue
            out = []
            for inst in insts:
                si = inst.get("sync_info")
                ow = (si or {}).get("on_wait") or []
                eng = inst.get("engine")
                if len(ow) > 1 and eng:
                    changed = True
                    for w in ow[:-1]:
                        _split_ctr[0] += 1
                        nop = {
                            "engine": eng,
                            "ins": [],
                            "outs": [],
                            "name": f"I-wsplit-{_split_ctr[0]}",
                            "opcode": "NoOp",
                            "sync_info": {"on_update": [], "on_wait": [w]},
                            "text_hint": "wait_split",
                        }
                        if inst.get("debug") is not None:
                            nop["debug"] = inst["debug"]
                        out.append(nop)
                    si["on_wait"] = [ow[-1]]
                out.append(inst)
            blk["instructions"] = out
    return json.dumps(m).encode() if changed else raw


_patched = False


def _install_patches():
    global _patched
    if _patched:
        return
    tile.TileContext._drain_and_barrier = _drain_and_barrier_split
    bass.Bass.to_json_bytes = _to_json_bytes_split
    # Calibrate the scheduler's cost model to measured HW rates: ACT and DVE
    # run slower than the stock model (per-op overheads), which otherwise
    # makes the static PE instruction stream stall on weight-production.
    from concourse.hw_specs import TRN2Spec

    TRN2Spec.CYCLE_T = {
        **TRN2Spec.CYCLE_T,
        mybir.EngineType.DVE: 1e9 / 0.96e9 * 1.4,
        mybir.EngineType.Activation: 1e9 / 1.2e9 * 1.9,
    }
    _patched = True


# ---------------------------------------------------------------------------
# Problem constants (hardcoded per contest rules)
# ---------------------------------------------------------------------------

T_FULL, O_FULL, I_FULL, G = 8192, 4096, 4096, 128
N_OSH = 8  # 8-way shard on out_features
O_SH = O_FULL // N_OSH  # 512
N_KC = I_FULL // 128  # 32 contraction chunks of 128
N_TT = T_FULL // 128  # 64 token tiles
K_SPLITS = [(0, 4), (4, 16), (16, 32)]  # phase-B k-ranges (PE warmup split)

f32 = mybir.dt.float32
f16 = mybir.dt.float16
i32 = mybir.dt.int32

AF = mybir.ActivationFunctionType
ALU = mybir.AluOpType


def build_program() -> bass.Bass:
    _install_patches()
    nc = bass.Bass()
    xt = nc.declare_dram_parameter("xt", [N_TT, 128, N_KC, 128], f16, isOutput=False)
    lgT = nc.declare_dram_parameter("lgT", [2, I_FULL, O_SH], f16, isOutput=False)
    mskT = nc.declare_dram_parameter("mskT", [I_FULL, O_SH], f16, isOutput=False)
    srep = nc.declare_dram_parameter("srep", [128, N_KC, O_SH], f16, isOutput=False)
    y = nc.declare_dram_parameter("y", [T_FULL, O_SH], f32, isOutput=True)

    lgT_t = lgT.rearrange("s (k p) o -> p k s o", p=128)  # [128, N_KC, 2, O_SH]
    mskT_t = mskT.rearrange("(k p) o -> p k o", p=128)  # [128, N_KC, O_SH]

    with tile.TileContext(nc) as tc:
        with (
            tc.tile_pool(name="persist", bufs=1) as persist,
            tc.tile_pool(name="wt", bufs=1) as wt_pool,
            tc.tile_pool(name="ysb", bufs=1) as ysb_pool,
            tc.tile_pool(name="wa", bufs=2) as wa,
            tc.tile_pool(name="xin", bufs=3) as xin,
            tc.tile_pool(name="yout", bufs=3) as yout,
            tc.tile_pool(name="psb", bufs=8, space="PSUM") as psb,
        ):
            s_exp = persist.tile([128, N_KC, O_SH], f16, tag="sexp")
            nc.sync.dma_start(out=s_exp, in_=srep[:, :, :])

            wT = [
                wt_pool.tile([128, O_SH], f16, tag=f"wT{kc}", name=f"wT{kc}")
                for kc in range(N_KC)
            ]
            y_sb = [
                ysb_pool.tile([128, O_SH], f16, tag=f"ysb{tt}", name=f"ysb{tt}")
                for tt in range(N_TT)
            ]

            def emit_a(kc):
                E = wa.tile([128, 2, O_SH], f16, tag="E", bufs=4)
                nc.sync.dma_start(out=E, in_=lgT_t[:, kc])
                M = wa.tile([128, O_SH], f16, tag="M", bufs=4)
                nc.sync.dma_start(out=M, in_=mskT_t[:, kc])
                Ef = E.rearrange("p s o -> p (s o)")
                nc.scalar.activation(out=Ef, in_=Ef, func=AF.Exp)
                # D' = e0 + e1 (f32); +1 folds into the Ln bias below
                D = wa.tile([128, O_SH], f32, tag="D")
                nc.gpsimd.tensor_tensor(out=D, in0=E[:, 0, :], in1=E[:, 1, :], op=ALU.add)
                N = wa.tile([128, O_SH], f16, tag="N")
                nc.gpsimd.tensor_tensor(out=N, in0=E[:, 1, :], in1=E[:, 0, :], op=ALU.subtract)
                Nm = wa.tile([128, O_SH], f16, tag="Nm")
                nc.gpsimd.tensor_tensor(out=Nm, in0=N, in1=M, op=ALU.mult)
                # r = exp(-ln(D'+1)) = 1/(1+e0+e1), fp16 out (Ln/Exp: one table)
                nc.scalar.activation(out=D, in_=D, func=AF.Ln, bias=1.0)
                r = wa.tile([128, O_SH], f16, tag="r")
                nc.scalar.activation(out=r, in_=D, func=AF.Exp, scale=-1.0)
                w1 = wa.tile([128, O_SH], f16, tag="w1")
                nc.vector.tensor_tensor(out=w1, in0=Nm, in1=r, op=ALU.mult)
                nc.vector.tensor_tensor(out=wT[kc], in0=w1, in1=s_exp[:, kc, :], op=ALU.mult)

            def emit_b(split, k0, k1):
                ks = k1 - k0
                for tt in range(N_TT):
                    xTt = xin.tile([128, ks, 128], f16, tag=f"x{split}")
                    nc.sync.dma_start(out=xTt, in_=xt[tt, :, k0:k1, :])
                    pb = psb.tile([128, O_SH], f32, tag="pb")
                    for k in range(ks):
                        nc.tensor.matmul(
                            out=pb,
                            lhsT=xTt[:, k, :],
                            rhs=wT[k0 + k],
                            start=(k == 0),
                            stop=(k == ks - 1),
                        )
                    if split == 0:
                        nc.vector.tensor_copy(out=y_sb[tt], in_=pb)
                    elif split == len(K_SPLITS) - 1:
                        yf = yout.tile([128, O_SH], f32, tag="yf")
                        nc.vector.tensor_tensor(out=yf, in0=pb, in1=y_sb[tt], op=ALU.add)
                        nc.sync.dma_start(out=y[tt * 128 : (tt + 1) * 128, :], in_=yf)
                    else:
                        nc.vector.tensor_tensor(out=y_sb[tt], in0=pb, in1=y_sb[tt], op=ALU.add)

            for split, (k0, k1) in enumerate(K_SPLITS):
                for kc in range(k0, k1):
                    emit_a(kc)
                emit_b(split, k0, k1)

    return nc


_prog = None


def _get_program() -> bass.Bass:
    global _prog
    if _prog is None:
        _prog = build_program()
    return _prog


def _bf16_round(a: np.ndarray) -> np.ndarray:
    """Round f32 -> bf16 (RNE) -> f32, matching jax's bf16 cast."""
    u = np.ascontiguousarray(a, dtype=np.float32).view(np.uint32)
    r = ((u >> 16) & 1) + np.uint32(0x7FFF)
    return ((u + r) & np.uint32(0xFFFF0000)).view(np.float32)


def kernel(x, logits, scales, mask):
    nc = _get_program()
    x = np.asarray(x, dtype=np.float32)
    logits = np.asarray(logits, dtype=np.float32)
    scales = np.asarray(scales, dtype=np.float32)
    mask = np.asarray(mask)

    # x -> fp16 i-major tiles: xt[tt, p, kc, t] = x[tt*128+t, kc*128+p]
    xt = np.ascontiguousarray(
        x.astype(np.float16).reshape(N_TT, 128, N_KC, 128).transpose(0, 3, 2, 1)
    )
    s16 = _bf16_round(scales).astype(np.float16)  # [O, 32]

    in_maps = []
    for oq in range(N_OSH):
        o0, o1 = oq * O_SH, (oq + 1) * O_SH
        lgT = np.ascontiguousarray(
            logits[o0:o1].astype(np.float16).transpose(2, 1, 0)
        )  # [2, I, O_SH]
        mskT = np.ascontiguousarray(mask[o0:o1].T.astype(np.float16))  # [I, O_SH]
        srep = np.ascontiguousarray(
            np.broadcast_to(s16[o0:o1].T[None, :, :], (128, N_KC, O_SH))
        )  # [128, 32, O_SH]
        in_maps.append({"xt": xt, "lgT": lgT, "mskT": mskT, "srep": srep})

    res = run_bass_kernel_spmd(nc, in_maps, core_ids=list(range(8)))
    yf = np.empty((T_FULL, O_FULL), dtype=np.float32)
    for oq in range(N_OSH):
        yf[:, oq * O_SH : (oq + 1) * O_SH] = res.results[oq]["y"]
    return yf


# revision 43
# speedup vs baseline: 1.6720x; 1.0769x over previous
"""Trainium2 Bass kernel for nn_MirrorDescentLinear.

Reference computation:
    w[o,i] = (e1 - e0) / (1 + e0 + e1)            (softmax(+1) - softmax(-1))
    w *= bf16(scales)[o, i//128]                   (per-group scale)
    w *= mask[o,i]                                 (0/1 int mask)
    y = x @ w.T                                    (f32, [8192,4096]@[4096,4096].T)

Sharding (8 cores): tensor-parallel 8-way on out_features (O_SH=512/core),
tokens replicated. The host pre-transposes logits/mask to [I, O] layout and
x to i-major tiles (layout + fp16 cast only), so the whole weight pipeline
runs elementwise in the TRANSPOSED layout and produces wT[i, o] directly --
no PE transposes, no PSUM use in phase A. PE does nothing but the 2048
N=512 fp16 matmuls (the 437us roofline at 2.4GHz).

Per-core device program (fp16 math, f32 accumulation):
  phase A (weights, per 128-wide i-chunk kc): exp on ScalarE (fp16);
    D' = e0+e1 (f32), N = e1-e0, N*mask on GpSimd/Pool; 1/(1+D') as
    exp(-ln(D'+1)) on ScalarE (ln bias folds the +1, Ln/Exp share one
    activation table -> no table reloads); w = N*r*s on DVE (fp16 2x mode,
    the group scale is host-replicated across partitions so it is a packed
    operand). wT[kc] tiles are resident fp16 [128,512].
  phase B (matmul, 3 k-splits 4/12/16): per token-tile tt accumulate the
    split's k-range in PSUM; split 0 copies PSUM -> y_sb[tt] (fp16), split 1
    adds into y_sb, split 2 adds and stores f32 y. The early small split
    lets PE start ~20us in; SBUF partial-accumulation frees PSUM so all 8
    banks pipeline phase B.
"""

import json
import sys

sys.path.insert(0, "/opt/trn_rl_repo")

import numpy as np

import concourse.bass as bass
import concourse.tile as tile
from concourse import mybir
from concourse.bass_utils import run_bass_kernel_spmd
from concourse.tile_scheduler import N_PROCS
from concourse.vector_clock import ScopedClock, VectorClock

# ---------------------------------------------------------------------------
# Compatibility patches for the bundled walrus (accepts at most ONE sync wait
# per instruction; rejects any wait on Drain).
# ---------------------------------------------------------------------------


def _drain_and_barrier_split(self, tick_clock, wait_clock):
    g = tick_clock.global_clock
    for p in range(N_PROCS):
        tick = g.peek_next(p) - 1
        if tick <= 0:
            continue
        vc = VectorClock()
        vc.require_at_least(p, tick)
        nop = self.nc.sync.nop(nofuse=True, hint="tail_wait_split")
        wait_clock.add_sem_waits(nop.ins, ScopedClock({None: vc}))

    self.nc.sync.drain()

    self.nc.all_engine_barrier()
    assert self.sems is not None
    popped = self.nc._tile_sem_poison_stack.pop()
    assert popped is self._sem_poison
    self.nc.clear_and_free_semaphores(list(self.sems.allocated().values()))
    self.nc.all_engine_barrier()


_orig_to_json_bytes = bass.Bass.to_json_bytes
_split_ctr = [0]


def _to_json_bytes_split(self):
    raw = _orig_to_json_bytes(self)
    m = json.loads(raw)
    changed = False
    for fn in m.get("functions", []):
        for blk in fn.get("blocks", []):
            insts = blk.get("instructions")
            if not insts:
                continue
            out = []
            for inst in insts:
                si = inst.get("sync_info")
                ow = (si or {}).get("on_wait") or []
                eng = inst.get("engine")
                if len(ow) > 1 and eng:
                    changed = True
                    for w in ow[:-1]:
                        _split_ctr[0] += 1
                        nop = {
                            "engine": eng,
                            "ins": [],
                            "outs": [],
                            "name": f"I-wsplit-{_split_ctr[0]}",
                            "opcode": "NoOp",
                            "sync_info": {"on_update": [], "on_wait": [w]},
                            "text_hint": "wait_split",
                        }
                        if inst.get("debug") is not None:
                            nop["debug"] = inst["debug"]
                        out.append(nop)
                    si["on_wait"] = [ow[-1]]
                out.append(inst)
            blk["instructions"] = out
    return json.dumps(m).encode() if changed else raw


_patched = False


def _install_patches():
    global _patched
    if _patched:
        return
    tile.TileContext._drain_and_barrier = _drain_and_barrier_split
    bass.Bass.to_json_bytes = _to_json_bytes_split
    # Calibrate the scheduler's cost model to measured HW rates: ACT and DVE
    # run slower than the stock model (per-op overheads), which otherwise
    # makes the static PE instruction stream stall on weight-production.
    from concourse.hw_specs import TRN2Spec

    TRN2Spec.CYCLE_T = {
        **TRN2Spec.CYCLE_T,
        mybir.EngineType.DVE: 1e9 / 0.96e9 * 1.4,
        mybir.EngineType.Activation: 1e9 / 1.2e9 * 1.9,
    }
    _patched = True


# ---------------------------------------------------------------------------
# Problem constants (hardcoded per contest rules)
# ---------------------------------------------------------------------------

T_FULL, O_FULL, I_FULL, G = 8192, 4096, 4096, 128
N_OSH = 8  # 8-way shard on out_features
O_SH = O_FULL // N_OSH  # 512
N_KC = I_FULL // 128  # 32 contraction chunks of 128
N_TT = T_FULL // 128  # 64 token tiles
K_SPLITS = [(0, 4), (4, 16), (16, 32)]  # phase-B k-ranges (PE warmup)
SE_CHUNK = 4  # s_exp DMA chunk (kc per chunk)

f32 = mybir.dt.float32
f16 = mybir.dt.float16
i32 = mybir.dt.int32

AF = mybir.ActivationFunctionType
ALU = mybir.AluOpType


def build_program() -> bass.Bass:
    _install_patches()
    nc = bass.Bass()
    xt = nc.declare_dram_parameter("xt", [N_TT, 128, N_KC, 128], f16, isOutput=False)
    lgT = nc.declare_dram_parameter("lgT", [I_FULL, 2, O_SH], f16, isOutput=False)
    mskT = nc.declare_dram_parameter("mskT", [I_FULL, O_SH], f16, isOutput=False)
    srep = nc.declare_dram_parameter("srep", [128, N_KC, O_SH], f16, isOutput=False)
    y = nc.declare_dram_parameter("y", [T_FULL, O_SH], f32, isOutput=True)

    lgT_t = lgT.rearrange("(k p) s o -> p k s o", p=128)  # [128, N_KC, 2, O_SH]
    mskT_t = mskT.rearrange("(k p) o -> p k o", p=128)  # [128, N_KC, O_SH]

    xt_t = xt.rearrange("n p k t -> p n k t")  # [128, N_TT, N_KC, 128]
    y_t = y.rearrange("(n p) o -> p n o", p=128)  # [128, N_TT, O_SH]

    with tile.TileContext(nc) as tc:
        with (
            tc.tile_pool(name="persist", bufs=1) as persist,
            tc.tile_pool(name="wt", bufs=1) as wt_pool,
            tc.tile_pool(name="ysb", bufs=1) as ysb_pool,
            tc.tile_pool(name="wa", bufs=2) as wa,
            tc.tile_pool(name="xin", bufs=2) as xin,
            tc.tile_pool(name="yout", bufs=2) as yout,
            tc.tile_pool(name="psb", bufs=4, space="PSUM") as psb,
        ):
            n_se = N_KC // SE_CHUNK
            s_exp = [None] * n_se
            se_sent = [False] * n_se

            # paired wT tiles: wT2[j] holds kc = 2j, 2j+1
            wT2 = [
                wt_pool.tile([128, 2, O_SH], f16, tag=f"wT{j}", name=f"wT{j}")
                for j in range(N_KC // 2)
            ]

            def wT(kc):
                return wT2[kc // 2][:, kc % 2, :]

            y_sb = [
                ysb_pool.tile([128, 2, O_SH], f16, tag=f"ysb{tp}", name=f"ysb{tp}")
                for tp in range(N_TT // 2)
            ]


            def emit_se(c):
                if not se_sent[c]:
                    se_sent[c] = True
                    s_exp[c] = persist.tile(
                        [128, SE_CHUNK, O_SH], f16, tag="sexp", name=f"sexp{c}", bufs=2
                    )
                    nc.sync.dma_start(
                        out=s_exp[c], in_=srep[:, c * SE_CHUNK : (c + 1) * SE_CHUNK, :]
                    )

            def emit_a2(j):
                """Weight pipeline for the kc pair (2j, 2j+1).

                The reciprocal 1/(1+e0+e1) is routed by pair index: early
                pairs (j < 8) use DVE's iterative reciprocal (DVE is idle
                before the evacuation stream ramps, and this keeps ScalarE
                free to race ahead on the exps that gate split 1); late pairs
                use exp(-ln(D'+1)) on ScalarE, whose Ln bias folds the +1
                (Ln/Exp share one activation table -> no table reloads).
                """
                kc0 = 2 * j
                early = j < 2
                E = wa.tile([128, 2, 2, O_SH], f16, tag="E", bufs=2)
                nc.sync.dma_start(out=E, in_=lgT_t[:, kc0 : kc0 + 2])
                M = wa.tile([128, 2, O_SH], f16, tag="M", bufs=2)
                nc.sync.dma_start(out=M, in_=mskT_t[:, kc0 : kc0 + 2])
                emit_se(kc0 // SE_CHUNK)
                Ef = E.rearrange("p k s o -> p (k s o)")
                nc.scalar.activation(out=Ef, in_=Ef, func=AF.Exp)
                D = wa.tile([128, 2, O_SH], f32, tag="D")
                if early:
                    # D = 1 + e0 + e1 and its reciprocal, both on DVE
                    nc.vector.scalar_tensor_tensor(
                        out=D, in0=E[:, :, 0, :], scalar=1.0, in1=E[:, :, 1, :],
                        op0=ALU.add, op1=ALU.add,
                    )
                    r = wa.tile([128, 2, O_SH], f32, tag="rf")
                    nc.vector.reciprocal(out=r, in_=D)
                else:
                    # D' = e0 + e1 (Pool, first so ScalarE's ln can chain);
                    # the +1 folds into the Ln bias
                    nc.gpsimd.tensor_tensor(
                        out=D, in0=E[:, :, 0, :], in1=E[:, :, 1, :], op=ALU.add
                    )
                    Df = D.rearrange("p k o -> p (k o)")
                    nc.scalar.activation(out=Df, in_=Df, func=AF.Ln, bias=1.0)
                    r = wa.tile([128, 2, O_SH], f16, tag="r")
                    nc.scalar.activation(
                        out=r.rearrange("p k o -> p (k o)"), in_=Df, func=AF.Exp,
                        scale=-1.0,
                    )
                N = wa.tile([128, 2, O_SH], f16, tag="N")
                nc.gpsimd.tensor_tensor(
                    out=N, in0=E[:, :, 1, :], in1=E[:, :, 0, :], op=ALU.subtract
                )
                Nm = wa.tile([128, 2, O_SH], f16, tag="Nm")
                nc.gpsimd.tensor_tensor(out=Nm, in0=N, in1=M, op=ALU.mult)
                w1 = wa.tile([128, 2, O_SH], f16, tag="w1")
                if early:
                    nc.vector.tensor_tensor(out=w1, in0=Nm, in1=r, op=ALU.mult)
                else:
                    nc.gpsimd.tensor_tensor(out=w1, in0=Nm, in1=r, op=ALU.mult)
                c, off = divmod(kc0, SE_CHUNK)
                nc.vector.tensor_tensor(
                    out=wT2[j], in0=w1, in1=s_exp[c][:, off : off + 2, :], op=ALU.mult
                )

            def emit_b_pair(split, k0, k1, tt0):
                """Matmuls + paired-PSUM evac for token tiles tt0, tt0+1."""
                ks = k1 - k0
                last = split == len(K_SPLITS) - 1
                tp = tt0 // 2
                pb = psb.tile([128, 2, O_SH], f32, tag="pb")
                for n in range(2):
                    xTt = xin.tile([128, ks, 128], f16, tag=f"x{split}", bufs=4)
                    nc.sync.dma_start(out=xTt, in_=xt_t[:, tt0 + n, k0:k1, :])
                    for k in range(ks):
                        nc.tensor.matmul(
                            out=pb[:, n, :],
                            lhsT=xTt[:, k, :],
                            rhs=wT(k0 + k),
                            start=(k == 0),
                            stop=(k == ks - 1),
                        )
                if split == 0:
                    nc.vector.tensor_copy(out=y_sb[tp], in_=pb)
                elif last:
                    yf = yout.tile([128, 2, O_SH], f32, tag="yf", name="yf")
                    nc.vector.tensor_tensor(out=yf, in0=pb, in1=y_sb[tp], op=ALU.add)
                    nc.sync.dma_start(out=y_t[:, tt0 : tt0 + 2, :], in_=yf)
                else:
                    nc.vector.tensor_tensor(
                        out=y_sb[tp], in0=pb, in1=y_sb[tp], op=ALU.add
                    )

            def emit_b_wave0(k0, k1, wtt0):
                """k-major wave for split 0: 8 token tiles advance together
                k-by-k as weight pairs land, so PE starts on wT pair 0."""
                ks = k1 - k0
                xts, pbs = [], []
                for n in range(8):
                    xTt = xin.tile([128, ks, 128], f16, tag="x0", bufs=12)
                    nc.sync.dma_start(out=xTt, in_=xt_t[:, wtt0 + n, k0:k1, :])
                    xts.append(xTt)
                for tp in range(4):
                    pbs.append(psb.tile([128, 2, O_SH], f32, tag="pb", name="pb"))
                for k in range(ks):
                    for tp in range(4):
                        for n in range(2):
                            nc.tensor.matmul(
                                out=pbs[tp][:, n, :],
                                lhsT=xts[2 * tp + n][:, k, :],
                                rhs=wT(k0 + k),
                                start=(k == 0),
                                stop=(k == ks - 1),
                            )
                for tp in range(4):
                    nc.vector.tensor_copy(out=y_sb[wtt0 // 2 + tp], in_=pbs[tp])

            # --- interleaved emission -------------------------------------
            # A-pairs for split s+1 are spread through B-split s's tt loop so
            # DMA/engine issue order matches execution order.
            for j in range(K_SPLITS[0][1] // 2):
                emit_a2(j)
            for split, (k0, k1) in enumerate(K_SPLITS):
                if split + 1 < len(K_SPLITS):
                    a0, a1 = K_SPLITS[split + 1]
                    apairs = list(range(a0 // 2, (a1 + 1) // 2))
                else:
                    apairs = []
                ai = 0
                if split == 0:
                    n_w = N_TT // 8
                    stride = max(1, n_w // max(1, len(apairs)))
                    for w in range(n_w):
                        emit_b_wave0(k0, k1, 8 * w)
                        if ai < len(apairs) and w % stride == stride - 1:
                            with tc.high_priority(offset=200):
                                emit_a2(apairs[ai])
                            ai += 1
                else:
                    n_bp = N_TT // 2
                    stride = max(1, n_bp // max(1, len(apairs)))
                    for bp in range(n_bp):
                        emit_b_pair(split, k0, k1, 2 * bp)
                        if ai < len(apairs) and bp % stride == stride - 1:
                            with tc.high_priority(offset=200):
                                emit_a2(apairs[ai])
                            ai += 1
                while ai < len(apairs):
                    emit_a2(apairs[ai])
                    ai += 1

    return nc


_prog = None


def _get_program() -> bass.Bass:
    global _prog
    if _prog is None:
        _prog = build_program()
    return _prog


def _bf16_round(a: np.ndarray) -> np.ndarray:
    """Round f32 -> bf16 (RNE) -> f32, matching jax's bf16 cast."""
    u = np.ascontiguousarray(a, dtype=np.float32).view(np.uint32)
    r = ((u >> 16) & 1) + np.uint32(0x7FFF)
    return ((u + r) & np.uint32(0xFFFF0000)).view(np.float32)


def kernel(x, logits, scales, mask):
    nc = _get_program()
    x = np.asarray(x, dtype=np.float32)
    logits = np.asarray(logits, dtype=np.float32)
    scales = np.asarray(scales, dtype=np.float32)
    mask = np.asarray(mask)

    # x -> fp16 i-major tiles: xt[tt, p, kc, t] = x[tt*128+t, kc*128+p]
    xt = np.ascontiguousarray(
        x.astype(np.float16).reshape(N_TT, 128, N_KC, 128).transpose(0, 3, 2, 1)
    )
    s16 = _bf16_round(scales).astype(np.float16)  # [O, 32]

    in_maps = []
    for oq in range(N_OSH):
        o0, o1 = oq * O_SH, (oq + 1) * O_SH
        lgT = np.ascontiguousarray(
            logits[o0:o1].astype(np.float16).transpose(1, 2, 0)
        )  # [I, 2, O_SH]
        mskT = np.ascontiguousarray(mask[o0:o1].T.astype(np.float16))  # [I, O_SH]
        srep = np.ascontiguousarray(
            np.broadcast_to(s16[o0:o1].T[None, :, :], (128, N_KC, O_SH))
        )  # [128, 32, O_SH]
        in_maps.append({"xt": xt, "lgT": lgT, "mskT": mskT, "srep": srep})

    res = run_bass_kernel_spmd(nc, in_maps, core_ids=list(range(8)))
    yf = np.empty((T_FULL, O_FULL), dtype=np.float32)
    for oq in range(N_OSH):
        yf[:, oq * O_SH : (oq + 1) * O_SH] = res.results[oq]["y"]
    return yf


# revision 49
# speedup vs baseline: 1.6722x; 1.0001x over previous
"""Trainium2 Bass kernel for nn_MirrorDescentLinear.

Reference computation:
    w[o,i] = (e1 - e0) / (1 + e0 + e1)            (softmax(+1) - softmax(-1))
    w *= bf16(scales)[o, i//128]                   (per-group scale)
    w *= mask[o,i]                                 (0/1 int mask)
    y = x @ w.T                                    (f32, [8192,4096]@[4096,4096].T)

Sharding (8 cores): tensor-parallel 8-way on out_features (O_SH=512/core),
tokens replicated. The host pre-transposes logits/mask to [I, O] layout and
x to i-major tiles (layout + fp16 cast only), so the whole weight pipeline
runs elementwise in the TRANSPOSED layout and produces wT[i, o] directly --
no PE transposes, no PSUM use in phase A. PE does nothing but the 2048
N=512 fp16 matmuls (the ~437us roofline at 2.4GHz); sim/HW total 478.9us,
92% PE occupancy (vs 800.7us for the f32r transpose-based predecessor).

Per-core device program (fp16 math, f32 PSUM accumulation):
  phase A (weights, per pair of 128-wide i-chunks kc): one exp over both
    logit planes on ScalarE (fp16 in/out); N = e1-e0 and N*mask on
    GpSimd/Pool; r = 1/(1+e0+e1) via exp(-ln(D'+1)) on ScalarE for most
    pairs (the Ln bias folds the +1; Ln/Exp share one activation table ->
    no table reloads) but via DVE's iterative reciprocal for the first two
    pairs, which halves the first-weight latency since ScalarE then only
    runs exps at the head; w = (N*mask)*r on Pool, *s on DVE (fp16 2x
    packed -- the group scale is host-replicated across partitions).
    wT pair tiles are resident fp16 [128,2,512].
  phase B (matmul, k-splits 4/12/16): split 0 runs k-major waves of 8
    token tiles that advance k-by-k as weight pairs land (PE starts on the
    first pair, ~10us in); splits 1-2 run token-pair-major. PSUM pair
    tiles [128,2,512] (2 banks) let one DVE op evacuate 2 token tiles:
    split 0 copies PSUM -> y_sb (fp16), split 1 adds into y_sb, split 2
    adds and stores f32 y. Weight-pipeline ops are emitted interleaved
    into phase B at raised scheduler priority so production outruns the
    consumption edge.
"""

import json
import sys

sys.path.insert(0, "/opt/trn_rl_repo")

import numpy as np

import concourse.bass as bass
import concourse.tile as tile
from concourse import mybir
from concourse.bass_utils import run_bass_kernel_spmd
from concourse.tile_scheduler import N_PROCS
from concourse.vector_clock import ScopedClock, VectorClock

# ---------------------------------------------------------------------------
# Compatibility patches for the bundled walrus (accepts at most ONE sync wait
# per instruction; rejects any wait on Drain).
# ---------------------------------------------------------------------------


def _drain_and_barrier_split(self, tick_clock, wait_clock):
    g = tick_clock.global_clock
    for p in range(N_PROCS):
        tick = g.peek_next(p) - 1
        if tick <= 0:
            continue
        vc = VectorClock()
        vc.require_at_least(p, tick)
        nop = self.nc.sync.nop(nofuse=True, hint="tail_wait_split")
        wait_clock.add_sem_waits(nop.ins, ScopedClock({None: vc}))

    self.nc.sync.drain()

    self.nc.all_engine_barrier()
    assert self.sems is not None
    popped = self.nc._tile_sem_poison_stack.pop()
    assert popped is self._sem_poison
    self.nc.clear_and_free_semaphores(list(self.sems.allocated().values()))
    self.nc.all_engine_barrier()


_orig_to_json_bytes = bass.Bass.to_json_bytes
_split_ctr = [0]


def _to_json_bytes_split(self):
    raw = _orig_to_json_bytes(self)
    m = json.loads(raw)
    changed = False
    for fn in m.get("functions", []):
        for blk in fn.get("blocks", []):
            insts = blk.get("instructions")
            if not insts:
                continue
            out = []
            for inst in insts:
                si = inst.get("sync_info")
                ow = (si or {}).get("on_wait") or []
                eng = inst.get("engine")
                if len(ow) > 1 and eng:
                    changed = True
                    for w in ow[:-1]:
                        _split_ctr[0] += 1
                        nop = {
                            "engine": eng,
                            "ins": [],
                            "outs": [],
                            "name": f"I-wsplit-{_split_ctr[0]}",
                            "opcode": "NoOp",
                            "sync_info": {"on_update": [], "on_wait": [w]},
                            "text_hint": "wait_split",
                        }
                        if inst.get("debug") is not None:
                            nop["debug"] = inst["debug"]
                        out.append(nop)
                    si["on_wait"] = [ow[-1]]
                out.append(inst)
            blk["instructions"] = out
    return json.dumps(m).encode() if changed else raw


_patched = False


def _install_patches():
    global _patched
    if _patched:
        return
    tile.TileContext._drain_and_barrier = _drain_and_barrier_split
    bass.Bass.to_json_bytes = _to_json_bytes_split
    # Calibrate the scheduler's cost model to measured HW rates: ACT and DVE
    # run slower than the stock model (per-op overheads), which otherwise
    # makes the static PE instruction stream stall on weight-production.
    from concourse.hw_specs import TRN2Spec

    TRN2Spec.CYCLE_T = {
        **TRN2Spec.CYCLE_T,
        mybir.EngineType.DVE: 1e9 / 0.96e9 * 1.4,
        mybir.EngineType.Activation: 1e9 / 1.2e9 * 1.9,
    }
    _patched = True


# ---------------------------------------------------------------------------
# Problem constants (hardcoded per contest rules)
# ---------------------------------------------------------------------------

T_FULL, O_FULL, I_FULL, G = 8192, 4096, 4096, 128
N_OSH = 8  # 8-way shard on out_features
O_SH = O_FULL // N_OSH  # 512
N_KC = I_FULL // 128  # 32 contraction chunks of 128
N_TT = T_FULL // 128  # 64 token tiles
K_SPLITS = [(0, 4), (4, 16), (16, 32)]  # phase-B k-ranges (PE warmup)
SE_CHUNK = 4  # s_exp DMA chunk (kc per chunk)

f32 = mybir.dt.float32
f16 = mybir.dt.float16
i32 = mybir.dt.int32

AF = mybir.ActivationFunctionType
ALU = mybir.AluOpType


def build_program() -> bass.Bass:
    _install_patches()
    nc = bass.Bass()
    xt = nc.declare_dram_parameter("xt", [N_TT, 128, N_KC, 128], f16, isOutput=False)
    lgT = nc.declare_dram_parameter("lgT", [I_FULL, 2, O_SH], f16, isOutput=False)
    mskT = nc.declare_dram_parameter("mskT", [I_FULL, O_SH], f16, isOutput=False)
    srep = nc.declare_dram_parameter("srep", [128, N_KC, O_SH], f16, isOutput=False)
    y = nc.declare_dram_parameter("y", [T_FULL, O_SH], f32, isOutput=True)

    lgT_t = lgT.rearrange("(k p) s o -> p k s o", p=128)  # [128, N_KC, 2, O_SH]
    mskT_t = mskT.rearrange("(k p) o -> p k o", p=128)  # [128, N_KC, O_SH]

    xt_t = xt.rearrange("n p k t -> p n k t")  # [128, N_TT, N_KC, 128]
    y_t = y.rearrange("(n p) o -> p n o", p=128)  # [128, N_TT, O_SH]

    with tile.TileContext(nc) as tc:
        with (
            tc.tile_pool(name="persist", bufs=1) as persist,
            tc.tile_pool(name="wt", bufs=1) as wt_pool,
            tc.tile_pool(name="ysb", bufs=1) as ysb_pool,
            tc.tile_pool(name="wa", bufs=2) as wa,
            tc.tile_pool(name="xin", bufs=2) as xin,
            tc.tile_pool(name="yout", bufs=2) as yout,
            tc.tile_pool(name="psb", bufs=4, space="PSUM") as psb,
        ):
            n_se = N_KC // SE_CHUNK
            s_exp = [None] * n_se
            se_sent = [False] * n_se

            # paired wT tiles: wT2[j] holds kc = 2j, 2j+1
            wT2 = [
                wt_pool.tile([128, 2, O_SH], f16, tag=f"wT{j}", name=f"wT{j}")
                for j in range(N_KC // 2)
            ]

            def wT(kc):
                return wT2[kc // 2][:, kc % 2, :]

            y_sb = [
                ysb_pool.tile([128, 2, O_SH], f16, tag=f"ysb{tp}", name=f"ysb{tp}")
                for tp in range(N_TT // 2)
            ]


            def emit_se(c):
                if not se_sent[c]:
                    se_sent[c] = True
                    s_exp[c] = persist.tile(
                        [128, SE_CHUNK, O_SH], f16, tag="sexp", name=f"sexp{c}", bufs=2
                    )
                    nc.sync.dma_start(
                        out=s_exp[c], in_=srep[:, c * SE_CHUNK : (c + 1) * SE_CHUNK, :]
                    )

            def emit_a2(j):
                """Weight pipeline for the kc pair (2j, 2j+1).

                The reciprocal 1/(1+e0+e1) is routed by pair index: early
                pairs (j < 8) use DVE's iterative reciprocal (DVE is idle
                before the evacuation stream ramps, and this keeps ScalarE
                free to race ahead on the exps that gate split 1); late pairs
                use exp(-ln(D'+1)) on ScalarE, whose Ln bias folds the +1
                (Ln/Exp share one activation table -> no table reloads).
                """
                kc0 = 2 * j
                early = j < 2
                E = wa.tile([128, 2, 2, O_SH], f16, tag="E", bufs=2)
                nc.sync.dma_start(out=E, in_=lgT_t[:, kc0 : kc0 + 2])
                M = wa.tile([128, 2, O_SH], f16, tag="M", bufs=2)
                nc.sync.dma_start(out=M, in_=mskT_t[:, kc0 : kc0 + 2])
                emit_se(kc0 // SE_CHUNK)
                Ef = E.rearrange("p k s o -> p (k s o)")
                nc.scalar.activation(out=Ef, in_=Ef, func=AF.Exp)
                D = wa.tile([128, 2, O_SH], f32, tag="D")
                if early:
                    # D = 1 + e0 + e1 and its reciprocal, both on DVE
                    nc.vector.scalar_tensor_tensor(
                        out=D, in0=E[:, :, 0, :], scalar=1.0, in1=E[:, :, 1, :],
                        op0=ALU.add, op1=ALU.add,
                    )
                    r = wa.tile([128, 2, O_SH], f32, tag="rf")
                    nc.vector.reciprocal(out=r, in_=D)
                else:
                    # D' = e0 + e1 (Pool, first so ScalarE's ln can chain);
                    # the +1 folds into the Ln bias
                    nc.gpsimd.tensor_tensor(
                        out=D, in0=E[:, :, 0, :], in1=E[:, :, 1, :], op=ALU.add
                    )
                    Df = D.rearrange("p k o -> p (k o)")
                    nc.scalar.activation(out=Df, in_=Df, func=AF.Ln, bias=1.0)
                    r = wa.tile([128, 2, O_SH], f16, tag="r")
                    nc.scalar.activation(
                        out=r.rearrange("p k o -> p (k o)"), in_=Df, func=AF.Exp,
                        scale=-1.0,
                    )
                N = wa.tile([128, 2, O_SH], f16, tag="N")
                nc.gpsimd.tensor_tensor(
                    out=N, in0=E[:, :, 1, :], in1=E[:, :, 0, :], op=ALU.subtract
                )
                Nm = wa.tile([128, 2, O_SH], f16, tag="Nm")
                nc.gpsimd.tensor_tensor(out=Nm, in0=N, in1=M, op=ALU.mult)
                w1 = wa.tile([128, 2, O_SH], f16, tag="w1")
                if early:
                    nc.vector.tensor_tensor(out=w1, in0=Nm, in1=r, op=ALU.mult)
                else:
                    nc.gpsimd.tensor_tensor(out=w1, in0=Nm, in1=r, op=ALU.mult)
                c, off = divmod(kc0, SE_CHUNK)
                nc.vector.tensor_tensor(
                    out=wT2[j], in0=w1, in1=s_exp[c][:, off : off + 2, :], op=ALU.mult
                )

            def emit_b_pair(split, k0, k1, tt0):
                """Matmuls + paired-PSUM evac for token tiles tt0, tt0+1."""
                ks = k1 - k0
                last = split == len(K_SPLITS) - 1
                tp = tt0 // 2
                pb = psb.tile([128, 2, O_SH], f32, tag="pb")
                for n in range(2):
                    xTt = xin.tile([128, ks, 128], f16, tag=f"x{split}", bufs=(5 if k0 >= 16 else 4))
                    nc.sync.dma_start(out=xTt, in_=xt_t[:, tt0 + n, k0:k1, :])
                    for k in range(ks):
                        nc.tensor.matmul(
                            out=pb[:, n, :],
                            lhsT=xTt[:, k, :],
                            rhs=wT(k0 + k),
                            start=(k == 0),
                            stop=(k == ks - 1),
                        )
                if split == 0:
                    nc.vector.tensor_copy(out=y_sb[tp], in_=pb)
                elif last and tt0 >= N_TT - 4:
                    # tail: per-tt evac + store so the last DMA is small
                    for n in range(2):
                        yf = yout.tile([128, 1, O_SH], f32, tag="yf1", name="yf1")
                        nc.vector.tensor_tensor(
                            out=yf, in0=pb[:, n : n + 1, :], in1=y_sb[tp][:, n : n + 1, :],
                            op=ALU.add,
                        )
                        nc.sync.dma_start(out=y_t[:, tt0 + n : tt0 + n + 1, :], in_=yf)
                elif last:
                    yf = yout.tile([128, 2, O_SH], f32, tag="yf", name="yf")
                    nc.vector.tensor_tensor(out=yf, in0=pb, in1=y_sb[tp], op=ALU.add)
                    nc.sync.dma_start(out=y_t[:, tt0 : tt0 + 2, :], in_=yf)
                else:
                    nc.vector.tensor_tensor(
                        out=y_sb[tp], in0=pb, in1=y_sb[tp], op=ALU.add
                    )

            def emit_b_wave0(k0, k1, wtt0):
                """k-major wave for split 0: 8 token tiles advance together
                k-by-k as weight pairs land, so PE starts on wT pair 0."""
                ks = k1 - k0
                xts, pbs = [], []
                for n in range(8):
                    xTt = xin.tile([128, ks, 128], f16, tag="x0", bufs=12)
                    nc.sync.dma_start(out=xTt, in_=xt_t[:, wtt0 + n, k0:k1, :])
                    xts.append(xTt)
                for tp in range(4):
                    pbs.append(psb.tile([128, 2, O_SH], f32, tag="pb", name="pb"))
                for k in range(ks):
                    for tp in range(4):
                        for n in range(2):
                            nc.tensor.matmul(
                                out=pbs[tp][:, n, :],
                                lhsT=xts[2 * tp + n][:, k, :],
                                rhs=wT(k0 + k),
                                start=(k == 0),
                                stop=(k == ks - 1),
                            )
                for tp in range(4):
                    nc.vector.tensor_copy(out=y_sb[wtt0 // 2 + tp], in_=pbs[tp])

            # --- interleaved emission -------------------------------------
            # A-pairs for split s+1 are spread through B-split s's tt loop so
            # DMA/engine issue order matches execution order.
            for j in range(K_SPLITS[0][1] // 2):
                emit_a2(j)
            for split, (k0, k1) in enumerate(K_SPLITS):
                if split + 1 < len(K_SPLITS):
                    a0, a1 = K_SPLITS[split + 1]
                    apairs = list(range(a0 // 2, (a1 + 1) // 2))
                else:
                    apairs = []
                ai = 0
                if split == 0:
                    n_w = N_TT // 8
                    stride = max(1, n_w // max(1, len(apairs)))
                    for w in range(n_w):
                        emit_b_wave0(k0, k1, 8 * w)
                        if ai < len(apairs) and w % stride == stride - 1:
                            with tc.high_priority(offset=200):
                                emit_a2(apairs[ai])
                            ai += 1
                else:
                    n_bp = N_TT // 2
                    stride = max(1, n_bp // max(1, len(apairs)))
                    for bp in range(n_bp):
                        emit_b_pair(split, k0, k1, 2 * bp)
                        if ai < len(apairs) and bp % stride == stride - 1:
                            with tc.high_priority(offset=200):
                                emit_a2(apairs[ai])
                            ai += 1
                while ai < len(apairs):
                    emit_a2(apairs[ai])
                    ai += 1

    return nc


_prog = None


def _get_program() -> bass.Bass:
    global _prog
    if _prog is None:
        _prog = build_program()
    return _prog


def _bf16_round(a: np.ndarray) -> np.ndarray:
    """Round f32 -> bf16 (RNE) -> f32, matching jax's bf16 cast."""
    u = np.ascontiguousarray(a, dtype=np.float32).view(np.uint32)
    r = ((u >> 16) & 1) + np.uint32(0x7FFF)
    return ((u + r) & np.uint32(0xFFFF0000)).view(np.float32)


def kernel(x, logits, scales, mask):
    nc = _get_program()
    x = np.asarray(x, dtype=np.float32)
    logits = np.asarray(logits, dtype=np.float32)
    scales = np.asarray(scales, dtype=np.float32)
    mask = np.asarray(mask)

    # x -> fp16 i-major tiles: xt[tt, p, kc, t] = x[tt*128+t, kc*128+p]
    xt = np.ascontiguousarray(
        x.astype(np.float16).reshape(N_TT, 128, N_KC, 128).transpose(0, 3, 2, 1)
    )
    s16 = _bf16_round(scales).astype(np.float16)  # [O, 32]

    in_maps = []
    for oq in range(N_OSH):
        o0, o1 = oq * O_SH, (oq + 1) * O_SH
        lgT = np.ascontiguousarray(
            logits[o0:o1].astype(np.float16).transpose(1, 2, 0)
        )  # [I, 2, O_SH]
        mskT = np.ascontiguousarray(mask[o0:o1].T.astype(np.float16))  # [I, O_SH]
        srep = np.ascontiguousarray(
            np.broadcast_to(s16[o0:o1].T[None, :, :], (128, N_KC, O_SH))
        )  # [128, 32, O_SH]
        in_maps.append({"xt": xt, "lgT": lgT, "mskT": mskT, "srep": srep})

    res = run_bass_kernel_spmd(nc, in_maps, core_ids=list(range(8)))
    yf = np.empty((T_FULL, O_FULL), dtype=np.float32)
    for oq in range(N_OSH):
        yf[:, oq * O_SH : (oq + 1) * O_SH] = res.results[oq]["y"]
    return yf


# revision 62
# speedup vs baseline: 1.6788x; 1.0039x over previous
"""Trainium2 Bass kernel for nn_MirrorDescentLinear.

Reference computation:
    w[o,i] = (e1 - e0) / (1 + e0 + e1)            (softmax(+1) - softmax(-1))
    w *= bf16(scales)[o, i//128]                   (per-group scale)
    w *= mask[o,i]                                 (0/1 int mask)
    y = x @ w.T                                    (f32, [8192,4096]@[4096,4096].T)

Sharding (8 cores): tensor-parallel 8-way on out_features (O_SH=512/core),
tokens replicated. The host pre-transposes logits/mask to [I, O] layout and
x to i-major tiles (layout + fp16 cast only), so the whole weight pipeline
runs elementwise in the TRANSPOSED layout and produces wT[i, o] directly --
no PE transposes, no PSUM use in phase A. PE does nothing but the 2048
N=512 fp16 matmuls (the ~437us roofline at 2.4GHz); sim/HW total 477.0us,
93% PE occupancy (vs 800.7us for the f32r transpose-based predecessor).

Per-core device program (fp16 math, f32 PSUM accumulation):
  phase A (weights, per pair of 128-wide i-chunks kc): one exp over both
    logit planes on ScalarE (fp16 in/out); N = e1-e0 and N*mask on
    GpSimd/Pool; r = 1/(1+e0+e1) via exp(-ln(D'+1)) on ScalarE for most
    pairs (the Ln bias folds the +1; Ln/Exp share one activation table ->
    no table reloads) but via DVE's iterative reciprocal for the first two
    pairs, which halves the first-weight latency since ScalarE then only
    runs exps at the head (pair 0 additionally runs per-kc half-width
    stages end-to-end, so the first matmul issues ~10us in); w =
    (N*mask)*r on Pool, *s on DVE (fp16 2x packed -- the group scale is
    host-replicated across partitions). wT pair tiles are fp16 [128,2,512].
  phase B (matmul, k-splits 4/12/16): split 0 runs k-major waves of 8
    token tiles that advance k-by-k as weight pairs land (PE starts on the
    first pair, ~10us in); splits 1-2 run token-pair-major. PSUM pair
    tiles [128,2,512] (2 banks) let one DVE op evacuate 2 token tiles:
    split 0 copies PSUM -> y_sb (fp16), split 1 adds into y_sb, split 2
    adds and stores f32 y. Weight-pipeline ops are emitted interleaved
    into phase B at raised scheduler priority so production outruns the
    consumption edge.
"""

import json
import sys

sys.path.insert(0, "/opt/trn_rl_repo")

import numpy as np

import concourse.bass as bass
import concourse.tile as tile
from concourse import mybir
from concourse.bass_utils import run_bass_kernel_spmd
from concourse.tile_scheduler import N_PROCS
from concourse.vector_clock import ScopedClock, VectorClock

# ---------------------------------------------------------------------------
# Compatibility patches for the bundled walrus (accepts at most ONE sync wait
# per instruction; rejects any wait on Drain).
# ---------------------------------------------------------------------------


def _drain_and_barrier_split(self, tick_clock, wait_clock):
    g = tick_clock.global_clock
    for p in range(N_PROCS):
        tick = g.peek_next(p) - 1
        if tick <= 0:
            continue
        vc = VectorClock()
        vc.require_at_least(p, tick)
        nop = self.nc.sync.nop(nofuse=True, hint="tail_wait_split")
        wait_clock.add_sem_waits(nop.ins, ScopedClock({None: vc}))

    self.nc.sync.drain()

    self.nc.all_engine_barrier()
    assert self.sems is not None
    popped = self.nc._tile_sem_poison_stack.pop()
    assert popped is self._sem_poison
    self.nc.clear_and_free_semaphores(list(self.sems.allocated().values()))
    self.nc.all_engine_barrier()


_orig_to_json_bytes = bass.Bass.to_json_bytes
_split_ctr = [0]


def _to_json_bytes_split(self):
    raw = _orig_to_json_bytes(self)
    m = json.loads(raw)
    changed = False
    for fn in m.get("functions", []):
        for blk in fn.get("blocks", []):
            insts = blk.get("instructions")
            if not insts:
                continue
            out = []
            for inst in insts:
                si = inst.get("sync_info")
                ow = (si or {}).get("on_wait") or []
                eng = inst.get("engine")
                if len(ow) > 1 and eng:
                    changed = True
                    for w in ow[:-1]:
                        _split_ctr[0] += 1
                        nop = {
                            "engine": eng,
                            "ins": [],
                            "outs": [],
                            "name": f"I-wsplit-{_split_ctr[0]}",
                            "opcode": "NoOp",
                            "sync_info": {"on_update": [], "on_wait": [w]},
                            "text_hint": "wait_split",
                        }
                        if inst.get("debug") is not None:
                            nop["debug"] = inst["debug"]
                        out.append(nop)
                    si["on_wait"] = [ow[-1]]
                out.append(inst)
            blk["instructions"] = out
    return json.dumps(m).encode() if changed else raw


_patched = False


def _install_patches():
    global _patched
    if _patched:
        return
    tile.TileContext._drain_and_barrier = _drain_and_barrier_split
    bass.Bass.to_json_bytes = _to_json_bytes_split
    # Calibrate the scheduler's cost model to measured HW rates: ACT and DVE
    # run slower than the stock model (per-op overheads), which otherwise
    # makes the static PE instruction stream stall on weight-production.
    from concourse.hw_specs import TRN2Spec

    TRN2Spec.CYCLE_T = {
        **TRN2Spec.CYCLE_T,
        mybir.EngineType.DVE: 1e9 / 0.96e9 * 1.4,
        mybir.EngineType.Activation: 1e9 / 1.2e9 * 1.9,
    }
    _patched = True


# ---------------------------------------------------------------------------
# Problem constants (hardcoded per contest rules)
# ---------------------------------------------------------------------------

T_FULL, O_FULL, I_FULL, G = 8192, 4096, 4096, 128
N_OSH = 8  # 8-way shard on out_features
O_SH = O_FULL // N_OSH  # 512
N_KC = I_FULL // 128  # 32 contraction chunks of 128
N_TT = T_FULL // 128  # 64 token tiles
K_SPLITS = [(0, 4), (4, 16), (16, 32)]  # phase-B k-ranges (PE warmup)
SE_CHUNK = 4  # s_exp DMA chunk (kc per chunk)

f32 = mybir.dt.float32
f16 = mybir.dt.float16
i32 = mybir.dt.int32

AF = mybir.ActivationFunctionType
ALU = mybir.AluOpType


def build_program() -> bass.Bass:
    _install_patches()
    nc = bass.Bass()
    xt = nc.declare_dram_parameter("xt", [N_TT, 128, N_KC, 128], f16, isOutput=False)
    lgT = nc.declare_dram_parameter("lgT", [I_FULL, 2, O_SH], f16, isOutput=False)
    mskT = nc.declare_dram_parameter("mskT", [I_FULL, O_SH], f16, isOutput=False)
    srep = nc.declare_dram_parameter("srep", [128, N_KC, O_SH], f16, isOutput=False)
    y = nc.declare_dram_parameter("y", [T_FULL, O_SH], f32, isOutput=True)

    lgT_t = lgT.rearrange("(k p) s o -> p k s o", p=128)  # [128, N_KC, 2, O_SH]
    mskT_t = mskT.rearrange("(k p) o -> p k o", p=128)  # [128, N_KC, O_SH]

    xt_t = xt.rearrange("n p k t -> p n k t")  # [128, N_TT, N_KC, 128]
    y_t = y.rearrange("(n p) o -> p n o", p=128)  # [128, N_TT, O_SH]

    with tile.TileContext(nc) as tc:
        with (
            tc.tile_pool(name="persist", bufs=1) as persist,
            tc.tile_pool(name="wt", bufs=1) as wt_pool,
            tc.tile_pool(name="ysb", bufs=1) as ysb_pool,
            tc.tile_pool(name="wa", bufs=2) as wa,
            tc.tile_pool(name="xin", bufs=2) as xin,
            tc.tile_pool(name="yout", bufs=2) as yout,
            tc.tile_pool(name="psb", bufs=4, space="PSUM") as psb,
        ):
            n_se = N_KC // SE_CHUNK
            s_exp = [None] * n_se
            se_sent = [False] * n_se

            # paired wT tiles: wT2[j] holds kc = 2j, 2j+1
            wT2 = [
                wt_pool.tile([128, 2, O_SH], f16, tag=f"wT{j}", name=f"wT{j}")
                for j in range(N_KC // 2)
            ]

            def wT(kc):
                return wT2[kc // 2][:, kc % 2, :]

            y_sb = [
                ysb_pool.tile([128, 2, O_SH], f16, tag=f"ysb{tp}", name=f"ysb{tp}")
                for tp in range(N_TT // 2)
            ]


            def emit_se(c):
                if not se_sent[c]:
                    se_sent[c] = True
                    s_exp[c] = persist.tile(
                        [128, SE_CHUNK, O_SH], f16, tag="sexp", name=f"sexp{c}", bufs=2
                    )
                    nc.sync.dma_start(
                        out=s_exp[c], in_=srep[:, c * SE_CHUNK : (c + 1) * SE_CHUNK, :]
                    )

            def emit_a2(j):
                """Weight pipeline for the kc pair (2j, 2j+1).

                The reciprocal 1/(1+e0+e1) is routed by pair index: early
                pairs (j < 8) use DVE's iterative reciprocal (DVE is idle
                before the evacuation stream ramps, and this keeps ScalarE
                free to race ahead on the exps that gate split 1); late pairs
                use exp(-ln(D'+1)) on ScalarE, whose Ln bias folds the +1
                (Ln/Exp share one activation table -> no table reloads).
                """
                kc0 = 2 * j
                early = j < 2
                E = wa.tile([128, 2, 2, O_SH], f16, tag="E", bufs=2)
                halves = range(2) if j == 0 else [slice(None)]
                for h in halves:
                    hh = slice(h, h + 1) if isinstance(h, int) else h
                    nc.sync.dma_start(out=E[:, hh], in_=lgT_t[:, kc0 : kc0 + 2][:, hh])
                M = wa.tile([128, 2, O_SH], f16, tag="M", bufs=2)
                nc.sync.dma_start(out=M, in_=mskT_t[:, kc0 : kc0 + 2])
                emit_se(kc0 // SE_CHUNK)
                for h in halves:
                    hh = slice(h, h + 1) if isinstance(h, int) else h
                    Ef = E[:, hh].rearrange("p k s o -> p (k s o)")
                    nc.scalar.activation(out=Ef, in_=Ef, func=AF.Exp)
                D = wa.tile([128, 2, O_SH], f32, tag="D")
                if early:
                    # D = 1 + e0 + e1 and its reciprocal, both on DVE.  Pair 0
                    # runs per-kc half-width stages end-to-end (including the
                    # logits DMA and exp above) to halve first-weight latency:
                    # the k-major wave can start on wT[kc=0] alone.
                    r = wa.tile([128, 2, O_SH], f32, tag="rf")
                    for h in halves:
                        hh = slice(h, h + 1) if isinstance(h, int) else h
                        nc.vector.scalar_tensor_tensor(
                            out=D[:, hh], in0=E[:, hh, 0, :], scalar=1.0,
                            in1=E[:, hh, 1, :], op0=ALU.add, op1=ALU.add,
                        )
                        nc.vector.reciprocal(out=r[:, hh], in_=D[:, hh])
                else:
                    # D' = e0 + e1 (Pool, first so ScalarE's ln can chain);
                    # the +1 folds into the Ln bias
                    nc.gpsimd.tensor_tensor(
                        out=D, in0=E[:, :, 0, :], in1=E[:, :, 1, :], op=ALU.add
                    )
                    Df = D.rearrange("p k o -> p (k o)")
                    nc.scalar.activation(out=Df, in_=Df, func=AF.Ln, bias=1.0)
                    r = wa.tile([128, 2, O_SH], f16, tag="r")
                    nc.scalar.activation(
                        out=r.rearrange("p k o -> p (k o)"), in_=Df, func=AF.Exp,
                        scale=-1.0,
                    )
                N = wa.tile([128, 2, O_SH], f16, tag="N")
                Nm = wa.tile([128, 2, O_SH], f16, tag="Nm")
                for h in halves:
                    hh = slice(h, h + 1) if isinstance(h, int) else h
                    nc.gpsimd.tensor_tensor(
                        out=N[:, hh], in0=E[:, hh, 1, :], in1=E[:, hh, 0, :],
                        op=ALU.subtract,
                    )
                    nc.gpsimd.tensor_tensor(
                        out=Nm[:, hh], in0=N[:, hh], in1=M[:, hh], op=ALU.mult
                    )
                w1 = wa.tile([128, 2, O_SH], f16, tag="w1")
                c, off = divmod(kc0, SE_CHUNK)
                for h in halves:
                    hh = slice(h, h + 1) if isinstance(h, int) else h
                    oo = slice(off + h, off + h + 1) if isinstance(h, int) else slice(off, off + 2)
                    if early:
                        nc.vector.tensor_tensor(
                            out=w1[:, hh], in0=Nm[:, hh], in1=r[:, hh], op=ALU.mult
                        )
                    else:
                        nc.gpsimd.tensor_tensor(
                            out=w1[:, hh], in0=Nm[:, hh], in1=r[:, hh], op=ALU.mult
                        )
                    nc.vector.tensor_tensor(
                        out=wT2[j][:, hh], in0=w1[:, hh], in1=s_exp[c][:, oo, :],
                        op=ALU.mult,
                    )

            def emit_b_pair(split, k0, k1, tt0):
                """Matmuls + paired-PSUM evac for token tiles tt0, tt0+1."""
                ks = k1 - k0
                last = split == len(K_SPLITS) - 1
                tp = tt0 // 2
                pb = psb.tile([128, 2, O_SH], f32, tag="pb")
                for n in range(2):
                    xTt = xin.tile([128, ks, 128], f16, tag=f"x{split}", bufs=(5 if k0 >= 16 else 4))
                    nc.sync.dma_start(out=xTt, in_=xt_t[:, tt0 + n, k0:k1, :])
                    for k in range(ks):
                        nc.tensor.matmul(
                            out=pb[:, n, :],
                            lhsT=xTt[:, k, :],
                            rhs=wT(k0 + k),
                            start=(k == 0),
                            stop=(k == ks - 1),
                        )
                if split == 0:
                    nc.vector.tensor_copy(out=y_sb[tp], in_=pb)
                elif last and tt0 >= N_TT - 4:
                    # tail: per-tt evac + store so the last DMA is small
                    for n in range(2):
                        yf = yout.tile([128, 1, O_SH], f32, tag="yf1", name="yf1")
                        nc.vector.tensor_tensor(
                            out=yf, in0=pb[:, n : n + 1, :], in1=y_sb[tp][:, n : n + 1, :],
                            op=ALU.add,
                        )
                        nc.sync.dma_start(out=y_t[:, tt0 + n : tt0 + n + 1, :], in_=yf)
                elif last:
                    yf = yout.tile([128, 2, O_SH], f32, tag="yf", name="yf")
                    nc.vector.tensor_tensor(out=yf, in0=pb, in1=y_sb[tp], op=ALU.add)
                    nc.sync.dma_start(out=y_t[:, tt0 : tt0 + 2, :], in_=yf)
                else:
                    nc.vector.tensor_tensor(
                        out=y_sb[tp], in0=pb, in1=y_sb[tp], op=ALU.add
                    )

            def emit_b_wave0(k0, k1, wtt0):
                """k-major wave for split 0: 8 token tiles advance together
                k-by-k as weight pairs land, so PE starts on wT pair 0."""
                ks = k1 - k0
                xts, pbs = [], []
                for n in range(8):
                    xTt = xin.tile([128, ks, 128], f16, tag="x0", bufs=12)
                    nc.sync.dma_start(out=xTt, in_=xt_t[:, wtt0 + n, k0:k1, :])
                    xts.append(xTt)
                for tp in range(4):
                    pbs.append(psb.tile([128, 2, O_SH], f32, tag="pb", name="pb"))
                for k in range(ks):
                    for tp in range(4):
                        for n in range(2):
                            nc.tensor.matmul(
                                out=pbs[tp][:, n, :],
                                lhsT=xts[2 * tp + n][:, k, :],
                                rhs=wT(k0 + k),
                                start=(k == 0),
                                stop=(k == ks - 1),
                            )
                for tp in range(4):
                    nc.vector.tensor_copy(out=y_sb[wtt0 // 2 + tp], in_=pbs[tp])

            # --- interleaved emission -------------------------------------
            # A-pairs for split s+1 are spread through B-split s's tt loop so
            # DMA/engine issue order matches execution order.
            for j in range(K_SPLITS[0][1] // 2):
                emit_a2(j)
            for split, (k0, k1) in enumerate(K_SPLITS):
                if split + 1 < len(K_SPLITS):
                    a0, a1 = K_SPLITS[split + 1]
                    apairs = list(range(a0 // 2, (a1 + 1) // 2))
                else:
                    apairs = []
                ai = 0
                if split == 0:
                    n_w = N_TT // 8
                    stride = max(1, n_w // max(1, len(apairs)))
                    for w in range(n_w):
                        emit_b_wave0(k0, k1, 8 * w)
                        if ai < len(apairs) and w % stride == stride - 1:
                            with tc.high_priority(offset=200):
                                emit_a2(apairs[ai])
                            ai += 1
                else:
                    n_bp = N_TT // 2
                    stride = max(1, n_bp // max(1, len(apairs)))
                    for bp in range(n_bp):
                        emit_b_pair(split, k0, k1, 2 * bp)
                        if ai < len(apairs) and bp % stride == stride - 1:
                            with tc.high_priority(offset=200):
                                emit_a2(apairs[ai])
                            ai += 1
                while ai < len(apairs):
                    emit_a2(apairs[ai])
                    ai += 1

    return nc


_prog = None


def _get_program() -> bass.Bass:
    global _prog
    if _prog is None:
        _prog = build_program()
    return _prog


def _bf16_round(a: np.ndarray) -> np.ndarray:
    """Round f32 -> bf16 (RNE) -> f32, matching jax's bf16 cast."""
    u = np.ascontiguousarray(a, dtype=np.float32).view(np.uint32)
    r = ((u >> 16) & 1) + np.uint32(0x7FFF)
    return ((u + r) & np.uint32(0xFFFF0000)).view(np.float32)


def kernel(x, logits, scales, mask):
    nc = _get_program()
    x = np.asarray(x, dtype=np.float32)
    logits = np.asarray(logits, dtype=np.float32)
    scales = np.asarray(scales, dtype=np.float32)
    mask = np.asarray(mask)

    # x -> fp16 i-major tiles: xt[tt, p, kc, t] = x[tt*128+t, kc*128+p]
    xt = np.ascontiguousarray(
        x.astype(np.float16).reshape(N_TT, 128, N_KC, 128).transpose(0, 3, 2, 1)
    )
    s16 = _bf16_round(scales).astype(np.float16)  # [O, 32]

    in_maps = []
    for oq in range(N_OSH):
        o0, o1 = oq * O_SH, (oq + 1) * O_SH
        lgT = np.ascontiguousarray(
            logits[o0:o1].astype(np.float16).transpose(1, 2, 0)
        )  # [I, 2, O_SH]
        mskT = np.ascontiguousarray(mask[o0:o1].T.astype(np.float16))  # [I, O_SH]
        srep = np.ascontiguousarray(
            np.broadcast_to(s16[o0:o1].T[None, :, :], (128, N_KC, O_SH))
        )  # [128, 32, O_SH]
        in_maps.append({"xt": xt, "lgT": lgT, "mskT": mskT, "srep": srep})

    res = run_bass_kernel_spmd(nc, in_maps, core_ids=list(range(8)))
    yf = np.empty((T_FULL, O_FULL), dtype=np.float32)
    for oq in range(N_OSH):
        yf[:, oq * O_SH : (oq + 1) * O_SH] = res.results[oq]["y"]
    return yf


# revision 72
# speedup vs baseline: 1.6876x; 1.0053x over previous
"""Trainium2 Bass kernel for nn_MirrorDescentLinear.

Reference computation:
    w[o,i] = (e1 - e0) / (1 + e0 + e1)            (softmax(+1) - softmax(-1))
    w *= bf16(scales)[o, i//128]                   (per-group scale)
    w *= mask[o,i]                                 (0/1 int mask)
    y = x @ w.T                                    (f32, [8192,4096]@[4096,4096].T)

Sharding (8 cores): tensor-parallel 8-way on out_features (O_SH=512/core),
tokens replicated. The host pre-transposes logits/mask to [I, O] layout and
x to i-major tiles (layout + fp16 cast only), so the whole weight pipeline
runs elementwise in the TRANSPOSED layout and produces wT[i, o] directly --
no PE transposes, no PSUM use in phase A. PE does nothing but the 2048
N=512 fp16 matmuls (the ~437us roofline at 2.4GHz); sim/HW total 474.5us,
94% PE occupancy (vs 800.7us for the f32r transpose-based predecessor).

Per-core device program (fp16 math, f32 PSUM accumulation):
  phase A (weights, per pair of 128-wide i-chunks kc): one exp over both
    logit planes on ScalarE (fp16 in/out); N = e1-e0 and N*mask on
    GpSimd/Pool; r = 1/(1+e0+e1) via exp(-ln(D'+1)) on ScalarE for most
    pairs (the Ln bias folds the +1; Ln/Exp share one activation table ->
    no table reloads) but via DVE's iterative reciprocal for the first two
    pairs, which halves the first-weight latency since ScalarE then only
    runs exps at the head (pair 0 additionally runs per-kc half-width
    stages end-to-end, so the first matmul issues ~10us in); w =
    (N*mask)*r on Pool, *s on DVE (fp16 2x packed -- the group scale is
    host-replicated across partitions). wT pair tiles are fp16 [128,2,512].
  phase B (matmul, k-splits 4/12/16): split 0 runs k-major waves of 8
    token tiles that advance k-by-k as weight pairs land (PE starts on the
    first pair, ~10us in; each wave's x tiles arrive as ONE batched DMA to
    halve descriptor-generation serialization); splits 1-2 run
    token-pair-major. PSUM pair tiles [128,2,512] (2 banks) let one DVE op
    evacuate 2 token tiles:
    split 0 copies PSUM -> y_sb (fp16), split 1 adds into y_sb, split 2
    adds and stores f32 y. Weight-pipeline ops are emitted interleaved
    into phase B at raised scheduler priority so production outruns the
    consumption edge.
"""

import json
import sys

sys.path.insert(0, "/opt/trn_rl_repo")

import numpy as np

import concourse.bass as bass
import concourse.tile as tile
from concourse import mybir
from concourse.bass_utils import run_bass_kernel_spmd
from concourse.tile_scheduler import N_PROCS
from concourse.vector_clock import ScopedClock, VectorClock

# ---------------------------------------------------------------------------
# Compatibility patches for the bundled walrus (accepts at most ONE sync wait
# per instruction; rejects any wait on Drain).
# ---------------------------------------------------------------------------


def _drain_and_barrier_split(self, tick_clock, wait_clock):
    g = tick_clock.global_clock
    for p in range(N_PROCS):
        tick = g.peek_next(p) - 1
        if tick <= 0:
            continue
        vc = VectorClock()
        vc.require_at_least(p, tick)
        nop = self.nc.sync.nop(nofuse=True, hint="tail_wait_split")
        wait_clock.add_sem_waits(nop.ins, ScopedClock({None: vc}))

    self.nc.sync.drain()

    self.nc.all_engine_barrier()
    assert self.sems is not None
    popped = self.nc._tile_sem_poison_stack.pop()
    assert popped is self._sem_poison
    self.nc.clear_and_free_semaphores(list(self.sems.allocated().values()))
    self.nc.all_engine_barrier()


_orig_to_json_bytes = bass.Bass.to_json_bytes
_split_ctr = [0]


def _to_json_bytes_split(self):
    raw = _orig_to_json_bytes(self)
    m = json.loads(raw)
    changed = False
    for fn in m.get("functions", []):
        for blk in fn.get("blocks", []):
            insts = blk.get("instructions")
            if not insts:
                continue
            out = []
            for inst in insts:
                si = inst.get("sync_info")
                ow = (si or {}).get("on_wait") or []
                eng = inst.get("engine")
                if len(ow) > 1 and eng:
                    changed = True
                    for w in ow[:-1]:
                        _split_ctr[0] += 1
                        nop = {
                            "engine": eng,
                            "ins": [],
                            "outs": [],
                            "name": f"I-wsplit-{_split_ctr[0]}",
                            "opcode": "NoOp",
                            "sync_info": {"on_update": [], "on_wait": [w]},
                            "text_hint": "wait_split",
                        }
                        if inst.get("debug") is not None:
                            nop["debug"] = inst["debug"]
                        out.append(nop)
                    si["on_wait"] = [ow[-1]]
                out.append(inst)
            blk["instructions"] = out
    return json.dumps(m).encode() if changed else raw


_patched = False


def _install_patches():
    global _patched
    if _patched:
        return
    tile.TileContext._drain_and_barrier = _drain_and_barrier_split
    bass.Bass.to_json_bytes = _to_json_bytes_split
    # Calibrate the scheduler's cost model to measured HW rates: ACT and DVE
    # run slower than the stock model (per-op overheads), which otherwise
    # makes the static PE instruction stream stall on weight-production.
    from concourse.hw_specs import TRN2Spec

    TRN2Spec.CYCLE_T = {
        **TRN2Spec.CYCLE_T,
        mybir.EngineType.DVE: 1e9 / 0.96e9 * 1.4,
        mybir.EngineType.Activation: 1e9 / 1.2e9 * 1.9,
    }
    _patched = True


# ---------------------------------------------------------------------------
# Problem constants (hardcoded per contest rules)
# ---------------------------------------------------------------------------

T_FULL, O_FULL, I_FULL, G = 8192, 4096, 4096, 128
N_OSH = 8  # 8-way shard on out_features
O_SH = O_FULL // N_OSH  # 512
N_KC = I_FULL // 128  # 32 contraction chunks of 128
N_TT = T_FULL // 128  # 64 token tiles
K_SPLITS = [(0, 4), (4, 16), (16, 32)]  # phase-B k-ranges (PE warmup)
SE_CHUNK = 4  # s_exp DMA chunk (kc per chunk)

f32 = mybir.dt.float32
f16 = mybir.dt.float16
i32 = mybir.dt.int32

AF = mybir.ActivationFunctionType
ALU = mybir.AluOpType


def build_program() -> bass.Bass:
    _install_patches()
    nc = bass.Bass()
    xt = nc.declare_dram_parameter("xt", [N_TT, 128, N_KC, 128], f16, isOutput=False)
    lgT = nc.declare_dram_parameter("lgT", [I_FULL, 2, O_SH], f16, isOutput=False)
    mskT = nc.declare_dram_parameter("mskT", [I_FULL, O_SH], f16, isOutput=False)
    srep = nc.declare_dram_parameter("srep", [128, N_KC, O_SH], f16, isOutput=False)
    y = nc.declare_dram_parameter("y", [T_FULL, O_SH], f32, isOutput=True)

    lgT_t = lgT.rearrange("(k p) s o -> p k s o", p=128)  # [128, N_KC, 2, O_SH]
    mskT_t = mskT.rearrange("(k p) o -> p k o", p=128)  # [128, N_KC, O_SH]

    xt_t = xt.rearrange("n p k t -> p n k t")  # [128, N_TT, N_KC, 128]
    y_t = y.rearrange("(n p) o -> p n o", p=128)  # [128, N_TT, O_SH]

    with tile.TileContext(nc) as tc:
        with (
            tc.tile_pool(name="persist", bufs=1) as persist,
            tc.tile_pool(name="wt", bufs=1) as wt_pool,
            tc.tile_pool(name="ysb", bufs=1) as ysb_pool,
            tc.tile_pool(name="wa", bufs=2) as wa,
            tc.tile_pool(name="xin", bufs=2) as xin,
            tc.tile_pool(name="yout", bufs=2) as yout,
            tc.tile_pool(name="psb", bufs=4, space="PSUM") as psb,
        ):
            n_se = N_KC // SE_CHUNK
            s_exp = [None] * n_se
            se_sent = [False] * n_se

            # paired wT tiles: wT2[j] holds kc = 2j, 2j+1
            wT2 = [
                wt_pool.tile([128, 2, O_SH], f16, tag=f"wT{j}", name=f"wT{j}")
                for j in range(N_KC // 2)
            ]

            def wT(kc):
                return wT2[kc // 2][:, kc % 2, :]

            y_sb = [
                ysb_pool.tile([128, 2, O_SH], f16, tag=f"ysb{tp}", name=f"ysb{tp}")
                for tp in range(N_TT // 2)
            ]


            def emit_se(c):
                if not se_sent[c]:
                    se_sent[c] = True
                    s_exp[c] = persist.tile(
                        [128, SE_CHUNK, O_SH], f16, tag="sexp", name=f"sexp{c}", bufs=2
                    )
                    nc.sync.dma_start(
                        out=s_exp[c], in_=srep[:, c * SE_CHUNK : (c + 1) * SE_CHUNK, :]
                    )

            def emit_a2(j):
                """Weight pipeline for the kc pair (2j, 2j+1).

                The reciprocal 1/(1+e0+e1) is routed by pair index: early
                pairs (j < 8) use DVE's iterative reciprocal (DVE is idle
                before the evacuation stream ramps, and this keeps ScalarE
                free to race ahead on the exps that gate split 1); late pairs
                use exp(-ln(D'+1)) on ScalarE, whose Ln bias folds the +1
                (Ln/Exp share one activation table -> no table reloads).
                """
                kc0 = 2 * j
                early = j < 2
                E = wa.tile([128, 2, 2, O_SH], f16, tag="E", bufs=2)
                halves = range(2) if j == 0 else [slice(None)]
                for h in halves:
                    hh = slice(h, h + 1) if isinstance(h, int) else h
                    nc.sync.dma_start(out=E[:, hh], in_=lgT_t[:, kc0 : kc0 + 2][:, hh])
                M = wa.tile([128, 2, O_SH], f16, tag="M", bufs=2)
                nc.sync.dma_start(out=M, in_=mskT_t[:, kc0 : kc0 + 2])
                emit_se(kc0 // SE_CHUNK)
                for h in halves:
                    hh = slice(h, h + 1) if isinstance(h, int) else h
                    Ef = E[:, hh].rearrange("p k s o -> p (k s o)")
                    nc.scalar.activation(out=Ef, in_=Ef, func=AF.Exp)
                D = wa.tile([128, 2, O_SH], f32, tag="D")
                if early:
                    # D = 1 + e0 + e1 and its reciprocal, both on DVE.  Pair 0
                    # runs per-kc half-width stages end-to-end (including the
                    # logits DMA and exp above) to halve first-weight latency:
                    # the k-major wave can start on wT[kc=0] alone.
                    r = wa.tile([128, 2, O_SH], f32, tag="rf")
                    for h in halves:
                        hh = slice(h, h + 1) if isinstance(h, int) else h
                        nc.vector.scalar_tensor_tensor(
                            out=D[:, hh], in0=E[:, hh, 0, :], scalar=1.0,
                            in1=E[:, hh, 1, :], op0=ALU.add, op1=ALU.add,
                        )
                        nc.vector.reciprocal(out=r[:, hh], in_=D[:, hh])
                else:
                    # D' = e0 + e1 (Pool, first so ScalarE's ln can chain);
                    # the +1 folds into the Ln bias
                    nc.gpsimd.tensor_tensor(
                        out=D, in0=E[:, :, 0, :], in1=E[:, :, 1, :], op=ALU.add
                    )
                    Df = D.rearrange("p k o -> p (k o)")
                    nc.scalar.activation(out=Df, in_=Df, func=AF.Ln, bias=1.0)
                    r = wa.tile([128, 2, O_SH], f16, tag="r")
                    nc.scalar.activation(
                        out=r.rearrange("p k o -> p (k o)"), in_=Df, func=AF.Exp,
                        scale=-1.0,
                    )
                N = wa.tile([128, 2, O_SH], f16, tag="N")
                Nm = wa.tile([128, 2, O_SH], f16, tag="Nm")
                for h in halves:
                    hh = slice(h, h + 1) if isinstance(h, int) else h
                    nc.gpsimd.tensor_tensor(
                        out=N[:, hh], in0=E[:, hh, 1, :], in1=E[:, hh, 0, :],
                        op=ALU.subtract,
                    )
                    nc.gpsimd.tensor_tensor(
                        out=Nm[:, hh], in0=N[:, hh], in1=M[:, hh], op=ALU.mult
                    )
                w1 = wa.tile([128, 2, O_SH], f16, tag="w1")
                c, off = divmod(kc0, SE_CHUNK)
                for h in halves:
                    hh = slice(h, h + 1) if isinstance(h, int) else h
                    oo = slice(off + h, off + h + 1) if isinstance(h, int) else slice(off, off + 2)
                    if early:
                        nc.vector.tensor_tensor(
                            out=w1[:, hh], in0=Nm[:, hh], in1=r[:, hh], op=ALU.mult
                        )
                    else:
                        nc.gpsimd.tensor_tensor(
                            out=w1[:, hh], in0=Nm[:, hh], in1=r[:, hh], op=ALU.mult
                        )
                    nc.vector.tensor_tensor(
                        out=wT2[j][:, hh], in0=w1[:, hh], in1=s_exp[c][:, oo, :],
                        op=ALU.mult,
                    )

            def emit_b_pair(split, k0, k1, tt0):
                """Matmuls + paired-PSUM evac for token tiles tt0, tt0+1."""
                ks = k1 - k0
                last = split == len(K_SPLITS) - 1
                tp = tt0 // 2
                pb = psb.tile([128, 2, O_SH], f32, tag="pb")
                for n in range(2):
                    xTt = xin.tile(
                        [128, ks, 128], f16, tag=f"x{split}", bufs=(5 if k0 >= 16 else 4)
                    )
                    nc.sync.dma_start(out=xTt, in_=xt_t[:, tt0 + n, k0:k1, :])
                    for k in range(ks):
                        nc.tensor.matmul(
                            out=pb[:, n, :],
                            lhsT=xTt[:, k, :],
                            rhs=wT(k0 + k),
                            start=(k == 0),
                            stop=(k == ks - 1),
                        )
                if split == 0:
                    nc.vector.tensor_copy(out=y_sb[tp], in_=pb)
                elif last:
                    yf = yout.tile([128, 2, O_SH], f32, tag="yf", name="yf")
                    nc.vector.tensor_tensor(out=yf, in0=pb, in1=y_sb[tp], op=ALU.add)
                    nc.sync.dma_start(out=y_t[:, tt0 : tt0 + 2, :], in_=yf)
                else:
                    nc.vector.tensor_tensor(
                        out=y_sb[tp], in0=pb, in1=y_sb[tp], op=ALU.add
                    )

            def emit_b_wave0(k0, k1, wtt0):
                """k-major wave for split 0: 8 token tiles advance together
                k-by-k as weight pairs land, so PE starts on wT pair 0."""
                ks = k1 - k0
                pbs = []
                xw = xin.tile([128, 8, ks, 128], f16, tag="x0", bufs=2)
                nc.sync.dma_start(out=xw, in_=xt_t[:, wtt0 : wtt0 + 8, k0:k1, :])
                for tp in range(4):
                    pbs.append(psb.tile([128, 2, O_SH], f32, tag="pb", name="pb"))
                for k in range(ks):
                    for tp in range(4):
                        for n in range(2):
                            nc.tensor.matmul(
                                out=pbs[tp][:, n, :],
                                lhsT=xw[:, 2 * tp + n, k, :],
                                rhs=wT(k0 + k),
                                start=(k == 0),
                                stop=(k == ks - 1),
                            )
                for tp in range(4):
                    nc.vector.tensor_copy(out=y_sb[wtt0 // 2 + tp], in_=pbs[tp])

            # --- interleaved emission -------------------------------------
            # A-pairs for split s+1 are spread through B-split s's tt loop so
            # DMA/engine issue order matches execution order.
            for j in range(K_SPLITS[0][1] // 2):
                emit_a2(j)
            for split, (k0, k1) in enumerate(K_SPLITS):
                if split + 1 < len(K_SPLITS):
                    a0, a1 = K_SPLITS[split + 1]
                    apairs = list(range(a0 // 2, (a1 + 1) // 2))
                else:
                    apairs = []
                ai = 0
                if split == 0:
                    n_w = N_TT // 8
                    stride = max(1, n_w // max(1, len(apairs)))
                    for w in range(n_w):
                        emit_b_wave0(k0, k1, 8 * w)
                        if ai < len(apairs) and w % stride == stride - 1:
                            with tc.high_priority(offset=200):
                                emit_a2(apairs[ai])
                            ai += 1
                else:
                    n_bp = N_TT // 2
                    stride = max(1, n_bp // max(1, len(apairs)))
                    for bp in range(n_bp):
                        emit_b_pair(split, k0, k1, 2 * bp)
                        if ai < len(apairs) and bp % stride == stride - 1:
                            with tc.high_priority(offset=200):
                                emit_a2(apairs[ai])
                            ai += 1
                while ai < len(apairs):
                    emit_a2(apairs[ai])
                    ai += 1

    return nc


_prog = None


def _get_program() -> bass.Bass:
    global _prog
    if _prog is None:
        _prog = build_program()
    return _prog


def _bf16_round(a: np.ndarray) -> np.ndarray:
    """Round f32 -> bf16 (RNE) -> f32, matching jax's bf16 cast."""
    u = np.ascontiguousarray(a, dtype=np.float32).view(np.uint32)
    r = ((u >> 16) & 1) + np.uint32(0x7FFF)
    return ((u + r) & np.uint32(0xFFFF0000)).view(np.float32)


def kernel(x, logits, scales, mask):
    nc = _get_program()
    x = np.asarray(x, dtype=np.float32)
    logits = np.asarray(logits, dtype=np.float32)
    scales = np.asarray(scales, dtype=np.float32)
    mask = np.asarray(mask)

    # x -> fp16 i-major tiles: xt[tt, p, kc, t] = x[tt*128+t, kc*128+p]
    xt = np.ascontiguousarray(
        x.astype(np.float16).reshape(N_TT, 128, N_KC, 128).transpose(0, 3, 2, 1)
    )
    s16 = _bf16_round(scales).astype(np.float16)  # [O, 32]

    in_maps = []
    for oq in range(N_OSH):
        o0, o1 = oq * O_SH, (oq + 1) * O_SH
        lgT = np.ascontiguousarray(
            logits[o0:o1].astype(np.float16).transpose(1, 2, 0)
        )  # [I, 2, O_SH]
        mskT = np.ascontiguousarray(mask[o0:o1].T.astype(np.float16))  # [I, O_SH]
        srep = np.ascontiguousarray(
            np.broadcast_to(s16[o0:o1].T[None, :, :], (128, N_KC, O_SH))
        )  # [128, 32, O_SH]
        in_maps.append({"xt": xt, "lgT": lgT, "mskT": mskT, "srep": srep})

    res = run_bass_kernel_spmd(nc, in_maps, core_ids=list(range(8)))
    yf = np.empty((T_FULL, O_FULL), dtype=np.float32)
    for oq in range(N_OSH):
        yf[:, oq * O_SH : (oq + 1) * O_SH] = res.results[oq]["y"]
    return yf
